# revision 10
# baseline (speedup 1.0000x reference)
"""Trainium2 Bass kernel for nn_EnhancedFreqLCBlock.

Self-contained: accepts FULL inputs, returns FULL output.
Sharding: 8 cores = 2 batches x 4 quadrant Mamba blocks (expert parallel).
Each core: mask -> quadrant 2D-DCT -> channel LN -> Mamba (hardware
tensor_tensor_scan recurrence) -> residual -> quadrant IDCT contribution.
Host sums the 4 quadrant contributions per batch.
"""
import numpy as np

B, C, H, W = 2, 96, 128, 128
HQ, WQ = H // 2, W // 2          # 64, 64
L = HQ * WQ                      # 4096
D = 192                          # d_inner
S = 16                           # d_state
RK = 6                           # dt_rank
KCONV = 4
NCHUNK = 8
LC = L // NCHUNK                 # 512
NT = (D * S) // 128              # 24 scan partition-tiles
DA, DB = 128, 64                 # d split 192 = 128 + 64

_BUILT = {}


def _dct_mat(N):
    n = np.arange(N)
    M = np.cos(np.pi * (2 * n[None, :] + 1) * n[:, None] / (2 * N)) * np.sqrt(2.0 / N)
    M[0] *= 1.0 / np.sqrt(2.0)
    return M.astype(np.float32)


def _build_nc():
    import concourse.bacc as bacc
    import concourse.bass as bass
    import concourse.mybir as mybir
    import concourse.tile as tile

    f32 = mybir.dt.float32
    bf16 = mybir.dt.bfloat16
    AF = mybir.ActivationFunctionType
    OP = mybir.AluOpType
    AX = mybir.AxisListType
    ts = bass.ts

    nc = bacc.Bacc()

    # ---------------- DRAM I/O ----------------
    xb = nc.dram_tensor("xb", [C, H, W], bf16, kind="ExternalInput")
    xbh = nc.dram_tensor("xbh", [H, C, W], bf16, kind="ExternalInput")
    d_mhqT = nc.dram_tensor("mhqT", [H, HQ], bf16, kind="ExternalInput")
    d_mwqT = nc.dram_tensor("mwqT", [W, WQ], bf16, kind="ExternalInput")
    d_mhq = nc.dram_tensor("mhq", [HQ, H], bf16, kind="ExternalInput")
    d_mwq = nc.dram_tensor("mwq", [WQ, W], bf16, kind="ExternalInput")
    d_ident = nc.dram_tensor("ident", [128, 128], f32, kind="ExternalInput")
    d_inwT = nc.dram_tensor("inwT", [C, 2 * D], bf16, kind="ExternalInput")
    d_biasi = nc.dram_tensor("biasi", [D, 1], f32, kind="ExternalInput")
    d_biasz = nc.dram_tensor("biasz", [D, 1], f32, kind="ExternalInput")
    d_convw = nc.dram_tensor("convw", [D, KCONV], f32, kind="ExternalInput")
    d_convb = nc.dram_tensor("convb", [D, 1], f32, kind="ExternalInput")
    d_xpwT = nc.dram_tensor("xpwT", [D, RK + 2 * S], bf16, kind="ExternalInput")
    d_dtwT = nc.dram_tensor("dtwT", [RK, D], bf16, kind="ExternalInput")
    d_dtb = nc.dram_tensor("dtb", [D, 1], f32, kind="ExternalInput")
    d_acol = nc.dram_tensor("acol", [128, NT], f32, kind="ExternalInput")
    d_dp = nc.dram_tensor("dp", [D, 1], f32, kind="ExternalInput")
    d_outwT = nc.dram_tensor("outwT", [D, C], bf16, kind="ExternalInput")
    d_p0164 = nc.dram_tensor("p0164", [128, 128 * 8], bf16, kind="ExternalInput")
    d_s01 = nc.dram_tensor("s01", [S, 128], bf16, kind="ExternalInput")
    d_r01all = nc.dram_tensor("r01all", [128, 128 * 16], bf16, kind="ExternalInput")
    d_r01ball = nc.dram_tensor("r01ball", [128, 64 * 8], bf16, kind="ExternalInput")
    contrib = nc.dram_tensor("contrib", [H, C, W], bf16, kind="ExternalOutput")
    # per-chunk DRAM scratch for the dX row-replication round-trip
    d_dxs = nc.dram_tensor("dxscratch", [NCHUNK // 2, D, 2 * LC], bf16,
                           kind="Internal")

    with tile.TileContext(nc) as tc:
        consts = tc.alloc_tile_pool(name="consts", bufs=1)
        # issue the big input loads before the ~30 const loads: SP
        # dispatches DMAs in program order and the mask path gates startup
        pD_ = tc.alloc_tile_pool(name="pD", bufs=1)
        pB = tc.alloc_tile_pool(name="pB", bufs=1)
        pXH = tc.alloc_tile_pool(name="pXH", bufs=1)
        pA = tc.alloc_tile_pool(name="pA", bufs=1)
        xc = pA.tile([C, H * W], bf16)
        xb_c = xb.rearrange("c h w -> c (h w)")
        for k in (4, 0, 1, 2, 3, 5, 6, 7):
            nc.sync.dma_start(xc[:, ts(k, 2048)], xb_c[:, ts(k, 2048)])
        xh = pXH.tile([H, C * W], bf16)
        xh3 = xh.rearrange("h (c w) -> h c w", c=C)
        # c-chunked loads keep full 128-partition spans (4x the DMA rate of
        # h-chunked loads)
        for i in range(4):
            nc.sync.dma_start(xh3[:, ts(i, 24), :], xbh[:, ts(i, 24), :])

        def cload(dram, shape, dt=f32):
            t = consts.tile(shape, dt, name=f"c_{dram.name}")
            nc.sync.dma_start(t[:], dram[:])
            return t

        def cload2(dram, dt=f32):
            ta = consts.tile([DA] + list(dram.shape[1:]), dt, name=f"cA_{dram.name}")
            nc.sync.dma_start(ta[:], dram[0:DA])
            tb = consts.tile([DB] + list(dram.shape[1:]), dt, name=f"cB_{dram.name}")
            nc.sync.dma_start(tb[:], dram[DA:D])
            return ta, tb

        mhqT = cload(d_mhqT, [H, HQ], bf16)
        mwqT = cload(d_mwqT, [W, WQ], bf16)
        mhq = cload(d_mhq, [HQ, H], bf16)
        mwq64 = consts.tile([128, W], bf16, name="c_mwq64")
        nc.sync.dma_start(mwq64[64:128, :], d_mwq[:])
        ident = cload(d_ident, [128, 128])
        identb = consts.tile([C, C], bf16, name="identb")
        nc.vector.tensor_copy(identb[:], ident[0:C, 0:C])
        inwT = cload(d_inwT, [C, 2 * D], bf16)
        biasiA, biasiB = cload2(d_biasi)
        biaszA, biaszB = cload2(d_biasz)
        convwA, convwB = cload2(d_convw)
        convbA, convbB = cload2(d_convb)
        xpwTA, xpwTB = cload2(d_xpwT, bf16)
        dtwT = cload(d_dtwT, [RK, D], bf16)
        dtbA, dtbB = cload2(d_dtb)
        acol = cload(d_acol, [128, NT])
        dpA, dpB = cload2(d_dp)
        outwTA, outwTB = cload2(d_outwT, bf16)
        p0164 = cload(d_p0164, [128, 128 * 8], bf16)
        s01 = cload(d_s01, [S, 128], bf16)
        r01all = cload(d_r01all, [128, 128 * 16], bf16)
        r01ball = cload(d_r01ball, [128, 64 * 8], bf16)
        onesr = consts.tile([1, 128], f32)
        nc.vector.memset(onesr[:], 1.0)
        ones96b = consts.tile([C, 1], bf16)
        nc.vector.memset(ones96b[:], 1.0)
        eps64 = consts.tile([WQ, 1], f32)
        nc.vector.memset(eps64[:], 1e-5)

        # persistent psum pools (8 banks total: 4 + 2 + 2)
        pmm = tc.alloc_tile_pool(name="pmm", bufs=4, space="PSUM")
        ppy = tc.alloc_tile_pool(name="ppy", bufs=1, space="PSUM")
        ptp = tc.alloc_tile_pool(name="ptp", bufs=1, space="PSUM")

        def mmtile(p, n, nm):
            return pmm.tile([p, n], f32, name=nm, tag="mm")

        def tptile(p, n, nm, dt=f32):
            return ptp.tile([p, n], dt, name=nm, tag="tp")

        # =============== Phase A: mask ===============
        cpos = (H // 2) * W + (W // 2)
        center = xc[:, cpos:cpos + 1]                       # [96,1]
        cn_ps = tptile(1, 1, "cn_ps")
        nc.tensor.matmul(cn_ps[:], center, center, start=True, stop=True)
        s049 = pA.tile([1, 1], f32)
        nc.vector.tensor_scalar_mul(s049[:], cn_ps[:], 0.49)
        s049p = tptile(128, 1, "s049p")
        nc.tensor.matmul(s049p[:], onesr[:], s049[:], start=True, stop=True)
        s049b = pA.tile([128, 1], f32)
        nc.vector.tensor_copy(s049b[:], s049p[:])

        num_hw = pA.tile([128, 128], f32)
        ssq_hw = pA.tile([128, 128], f32)
        # num chunks pipeline through the 4-buffer mm psum pool; copies
        # alternate DVE/Scalar so neither engine throttles the matmul chain
        pSt = tc.alloc_tile_pool(name="pSt", bufs=4)
        for i in range(32):
            nps = mmtile(1, LC, "nps")
            nc.tensor.matmul(nps[:], center, xc[:, ts(i, LC)],
                             start=True, stop=True)
            nrow = pSt.tile([1, LC], f32, name="nrow")
            if i % 2 == 0:
                nc.vector.tensor_copy(nrow[:], nps[:])
            else:
                nc.scalar.copy(nrow[:], nps[:])
            nc.sync.dma_start(num_hw[ts(i, 4), :], nrow[:])
        pSt.release()
        # sum over c of x^2 computed in the [h, c, w] layout: square a
        # w-chunk on Scalar, reduce the (middle) c axis on DVE/GpSimd -> [h, w]
        pSq = tc.alloc_tile_pool(name="pSq", bufs=2)
        for i in range(4):
            sq = pSq.tile([128, C * 32], f32, name="sq")
            sq3 = sq.rearrange("h (c w) -> h c w", c=C)
            nc.scalar.activation(sq3[:, :, :], xh3[:, :, ts(i, 32)], AF.Square)
            nc.vector.tensor_reduce(
                ssq_hw[:, ts(i, 32)], sq3.transpose([0, 2, 1]), axis=AX.X,
                op=OP.add)
        pSq.release()

        thr = pA.tile([128, 128], f32)
        nc.scalar.activation(thr[:], ssq_hw[:], AF.Sqrt, bias=0.0, scale=s049b[:])
        nc.vector.tensor_scalar_add(thr[:], thr[:], 0.7e-6)
        mask_hw = pA.tile([128, 128], bf16)
        nc.vector.tensor_tensor(mask_hw[:], num_hw[:], thr[:], op=OP.is_ge)
        for i in range(4):
            nc.vector.tensor_tensor(
                 xh3[:, ts(i, 24), :], xh3[:, ts(i, 24), :],
                 mask_hw[:, None, :].broadcast_to([128, 24, 128]), op=OP.mult)
        pA.release()

        # =============== Phase B: forward DCT ===============
        # t2[w, c, hq] = sum_h x[h, c, w] * Mh_q[hq, h]  (per-c matmul, no
        # separate transpose pass)
        t2 = pB.tile([W, C * HQ], bf16)
        t2_3 = t2.rearrange("p (c q) -> p c q", c=C)
        for c0 in range(0, C, 8):
            tps = tptile(W, 8 * HQ, "tps")
            tps3 = tps.rearrange("p (c q) -> p c q", c=8)
            for k in range(8):
                nc.tensor.matmul(tps3[:, k, :], xh3[:, c0 + k, :], mhqT[:],
                                 start=True, stop=True)
            nc.scalar.activation(t2_3[:, c0:c0 + 8, :], tps3[:, :, :], AF.Copy)
        pXH.release()

        # xdqZ: rows 0:64 = xdq (base 0 for DVE pairing), rows 64:128 = Z
        xdqZ = pD_.tile([128, C * HQ], bf16)
        xdq3 = xdqZ.rearrange("p (c q) -> p c q", c=C)[0:HQ, :, :]
        Z3 = xdqZ.rearrange("p (c q) -> p c q", c=C)[HQ:128, :, :]
        xdq2 = xdqZ[0:HQ, :]
        Z2 = xdqZ[HQ:128, :]
        for i in range(12):
            xps = mmtile(WQ, LC, "xps")
            nc.tensor.matmul(xps[:], mwqT[:], t2[:, ts(i, LC)], start=True, stop=True)
            nc.any.tensor_copy(xdq2[:, ts(i, LC)], xps[:])
        pB.release()

        # =============== Phase C: LayerNorm over c ===============
        pG = tc.alloc_tile_pool(name="pG", bufs=1)
        pF = tc.alloc_tile_pool(name="pF", bufs=1)
        pE = tc.alloc_tile_pool(name="pE", bufs=1)
        pC = tc.alloc_tile_pool(name="pC", bufs=1)
        # LN stats chunked to the 12 stage-2 copy chunks (8 c's each) so the
        # reduces overlap the DCT matmuls instead of serializing the machine
        smu = pC.tile([WQ, HQ], f32)
        ssq2 = pC.tile([WQ, HQ], f32)
        xn = pC.tile([WQ, C * HQ], bf16)  # first used as xdq^2 scratch
        xn3s = xn.rearrange("p (c q) -> p c q", c=C)
        pPart = tc.alloc_tile_pool(name="pPart", bufs=3)
        for cb in range(C // 8):
            csl = bass.ds(cb * 8, 8)
            nc.vector.tensor_tensor(xn3s[:, csl, :], xdq3[:, csl, :],
                                    xdq3[:, csl, :], op=OP.mult)
            pm = pPart.tile([WQ, HQ], f32, name="pm")
            nc.vector.tensor_reduce(
                pm[:], xdq3[:, csl, :].transpose([0, 2, 1]), axis=AX.X,
                op=OP.add)
            psq = pPart.tile([WQ, HQ], f32, name="psq")
            nc.vector.tensor_reduce(
                psq[:], xn3s[:, csl, :].transpose([0, 2, 1]), axis=AX.X,
                op=OP.add)
            if cb == 0:
                nc.vector.tensor_copy(smu[:], pm[:])
                nc.vector.tensor_copy(ssq2[:], psq[:])
            else:
                nc.vector.tensor_tensor(smu[:], smu[:], pm[:], op=OP.add)
                nc.vector.tensor_tensor(ssq2[:], ssq2[:], psq[:], op=OP.add)
        pPart.release()
        mu = pC.tile([WQ, HQ], f32)
        nc.vector.tensor_scalar_mul(mu[:], smu[:], 1.0 / C)
        var = pC.tile([WQ, HQ], f32)
        nc.vector.tensor_scalar_mul(ssq2[:], ssq2[:], 1.0 / C)
        nc.vector.tensor_tensor(var[:], mu[:], mu[:], op=OP.mult)
        nc.vector.tensor_tensor(var[:], ssq2[:], var[:], op=OP.subtract)
        sd = pC.tile([WQ, HQ], f32)
        nc.scalar.activation(sd[:], var[:], AF.Sqrt, bias=eps64[:])
        inv = pC.tile([WQ, HQ], f32)
        nc.vector.reciprocal(inv[:], sd[:])
        # bf16 stats so the normalize runs at DVE 2x rate; chunk by hq so
        # the transposes start before the whole tensor is normalized
        mub = pC.tile([WQ, HQ], bf16)
        nc.vector.tensor_copy(mub[:], mu[:])
        invb = pC.tile([WQ, HQ], bf16)
        nc.vector.tensor_copy(invb[:], inv[:])
        xn3 = xn.rearrange("p (c q) -> p c q", c=C)
        xn_c = pE.tile([C, L], bf16)
        for h0 in range(0, HQ, 16):
            hsl = bass.ds(h0, 16)
            nc.vector.tensor_tensor(
                xn3[:, :, hsl], xdq3[:, :, hsl],
                mub[:, None, hsl].broadcast_to([WQ, C, 16]), op=OP.subtract)
            nc.vector.tensor_tensor(
                xn3[:, :, hsl], xn3[:, :, hsl],
                invb[:, None, hsl].broadcast_to([WQ, C, 16]), op=OP.mult)
            tps2 = tptile(C, 16 * WQ, "tps2", bf16)
            tps2_3 = tps2.rearrange("p (h q) -> p h q", h=16)
            for k in range(16):
                nc.tensor.matmul(tps2_3[:, k, :], xn3[:, :, h0 + k],
                                 identb[0:WQ, 0:WQ],
                                 is_transpose=True, start=True, stop=True)
            nc.scalar.activation(xn_c[:, h0 * WQ:(h0 + 16) * WQ], tps2[:], AF.Copy)
        pC.release()

        # =============== Phase D: in_proj + conv + silu ===============
        xiA = pF.tile([DA, KCONV - 1 + L], bf16)
        xiB = pF.tile([DB, KCONV - 1 + L], bf16)
        nc.vector.memset(xiA[:, 0:KCONV - 1], 0.0)
        nc.vector.memset(xiB[:, 0:KCONV - 1], 0.0)
        xi2A = pG.tile([DA, L], bf16)
        zsA = pG.tile([DA, L], bf16)
        xi2B_t = pG.tile([DB, L], bf16, name="xi2B_t")
        zsB_t = pG.tile([DB, L], bf16, name="zsB_t")
        xi2B = xi2B_t[:, :]
        zsB = zsB_t[:, :]
        for i in range(NCHUNK):
            ps0 = mmtile(128, LC, "ps0")
            nc.tensor.matmul(ps0[:], inwT[:, 0:128], xn_c[:, ts(i, LC)],
                             start=True, stop=True)
            ps1 = mmtile(128, LC, "ps1")
            nc.tensor.matmul(ps1[:], inwT[:, 128:256], xn_c[:, ts(i, LC)],
                             start=True, stop=True)
            ps2 = mmtile(128, LC, "ps2")
            nc.tensor.matmul(ps2[:], inwT[:, 256:384], xn_c[:, ts(i, LC)],
                             start=True, stop=True)
            o = KCONV - 1 + i * LC
            nc.scalar.activation(xiA[:, o:o + LC], ps0[:], AF.Identity,
                                 bias=biasiA[:])
            nc.vector.tensor_scalar_add(xiB[:, o:o + LC], ps1[0:64, :], biasiB[:])
            # z gate: silu(xz + bias) fused into one Scalar op from PSUM
            nc.scalar.activation(zsA[0:64, ts(i, LC)], ps1[64:128, :], AF.Silu,
                                 bias=biaszA[0:64, :])
            nc.scalar.activation(zsA[64:128, ts(i, LC)], ps2[0:64, :], AF.Silu,
                                 bias=biaszA[64:128, :])
            nc.scalar.activation(zsB[:, ts(i, LC)], ps2[64:128, :], AF.Silu,
                                 bias=biaszB[:])
        pE.release()

        for i in range(NCHUNK // 2):
            cc = bass.ds(i * 2 * LC, 2 * LC)
            nc.vector.tensor_scalar_mul(xi2A[:, cc], xiA[:, i * 2 * LC:(i + 1) * 2 * LC],
                                        convwA[:, 0:1])
            nc.vector.tensor_scalar_mul(xi2B[:, cc], xiB[:, i * 2 * LC:(i + 1) * 2 * LC],
                                        convwB[:, 0:1])
            for k in range(1, KCONV):
                nc.vector.scalar_tensor_tensor(
                    xi2A[:, cc], xiA[:, k + i * 2 * LC:k + (i + 1) * 2 * LC],
                    convwA[:, k:k + 1], xi2A[:, cc], op0=OP.mult, op1=OP.add)
                nc.vector.scalar_tensor_tensor(
                    xi2B[:, cc], xiB[:, k + i * 2 * LC:k + (i + 1) * 2 * LC],
                    convwB[:, k:k + 1], xi2B[:, cc], op0=OP.mult, op1=OP.add)
            # silu(conv + bias) fused in-place on Scalar
            nc.scalar.activation(xi2A[:, cc], xi2A[:, cc], AF.Silu,
                                 bias=convbA[:])
            nc.scalar.activation(xi2B[:, cc], xi2B[:, cc], AF.Silu,
                                 bias=convbB[:])
        pF.release()

        # ====== Phase F: scan loop, Lc=1024 double-chunks ======
        pT = tc.alloc_tile_pool(name="pT", bufs=3)
        hlast = pG.tile([128, NT], bf16)
        LCF = 2 * LC
        for i in range(NCHUNK // 2):
            dt_c = pT.tile([RK, LCF], bf16, name="dt_c", bufs=1)
            bm_c = pT.tile([S, LCF], bf16, name="bm_c", bufs=1)
            cm_c = pT.tile([S, LCF], bf16, name="cm_c", bufs=1)
            for h in range(2):
                off = i * LCF + h * LC
                sl = bass.ds(off, LC)
                dtps = mmtile(RK, LC, "dtps")
                nc.tensor.matmul(dtps[:], xpwTA[:, 0:RK], xi2A[:, sl],
                                 start=True, stop=False)
                nc.tensor.matmul(dtps[:], xpwTB[:, 0:RK], xi2B[:, sl],
                                 start=False, stop=True)
                nc.any.tensor_copy(dt_c[:, ts(h, LC)], dtps[:])
                bmps = mmtile(S, LC, "bmps")
                nc.tensor.matmul(bmps[:], xpwTA[:, RK:RK + S], xi2A[:, sl],
                                 start=True, stop=False)
                nc.tensor.matmul(bmps[:], xpwTB[:, RK:RK + S], xi2B[:, sl],
                                 start=False, stop=True)
                nc.any.tensor_copy(bm_c[:, ts(h, LC)], bmps[:])
                cmps = mmtile(S, LC, "cmps")
                nc.tensor.matmul(cmps[:], xpwTA[:, RK + S:RK + 2 * S],
                                 xi2A[:, sl], start=True, stop=False)
                nc.tensor.matmul(cmps[:], xpwTB[:, RK + S:RK + 2 * S],
                                 xi2B[:, sl], start=False, stop=True)
                nc.any.tensor_copy(cm_c[:, ts(h, LC)], cmps[:])
            deltaA = pT.tile([DA, LCF], bf16, name="deltaA")
            deltaB = pT.tile([DB, LCF], bf16, name="deltaB")
            for h in range(2):
                dtpA = mmtile(DA, LC, "dtpA")
                nc.tensor.matmul(dtpA[:], dtwT[:, 0:DA], dt_c[0:RK, ts(h, LC)],
                                 start=True, stop=True)
                nc.scalar.activation(deltaA[:, ts(h, LC)], dtpA[:], AF.Exp,
                                     bias=dtbA[:])
                dtpB = mmtile(DB, LC, "dtpB")
                nc.tensor.matmul(dtpB[:], dtwT[:, DA:D], dt_c[0:RK, ts(h, LC)],
                                 start=True, stop=True)
                nc.scalar.activation(deltaB[:, ts(h, LC)], dtpB[:], AF.Exp,
                                     bias=dtbB[:])
            nc.scalar.activation(deltaA[:], deltaA[:], AF.Ln, bias=1.0)
            nc.scalar.activation(deltaB[:], deltaB[:], AF.Ln, bias=1.0)
            dXA = pT.tile([DA, LCF], bf16, name="dXA")
            nc.vector.tensor_tensor(dXA[:], deltaA[:],
                                    xi2A[:, ts(i, LCF)], op=OP.mult)
            dXB = pT.tile([DB, LCF], bf16, name="dXB")
            nc.vector.tensor_tensor(dXB[:], deltaB[:],
                                    xi2B[:, ts(i, LCF)], op=OP.mult)

            brep = pT.tile([128, LCF], bf16, name="brep")
            crep = pT.tile([128, LCF], bf16, name="crep")
            for h in range(2):
                brep_ps = mmtile(128, LC, "brep_ps")
                nc.tensor.matmul(brep_ps[:], s01[:], bm_c[:, ts(h, LC)],
                                 start=True, stop=True)
                nc.any.tensor_copy(brep[:, ts(h, LC)], brep_ps[:])
                crep_ps = mmtile(128, LC, "crep_ps")
                nc.tensor.matmul(crep_ps[:], s01[:], cm_c[:, ts(h, LC)],
                                 start=True, stop=True)
                nc.any.tensor_copy(crep[:, ts(h, LC)], crep_ps[:])

            ypsA0 = ppy.tile([128, LC], f32, name="ypsA0", tag="ypsA0")
            ypsA1 = ppy.tile([128, LC], f32, name="ypsA1", tag="ypsA1")
            ypsBp = ppy.tile([128, LC], f32, name="ypsBp", tag="ypsBp")
            ypsB0 = ypsBp[0:DB, :]
            ypsB1 = ypsBp[DB:128, :]
            for j in range(NT):
                jj = j if j < 16 else j - 16
                if j < 8:
                    dsl, xsl = deltaA[0:64, :], dXA[0:64, :]
                    psel = p0164[0:64, ts(jj % 8, 128)]
                elif j < 16:
                    dsl, xsl = deltaA[64:128, :], dXA[64:128, :]
                    psel = p0164[64:128, ts(jj % 8, 128)]
                else:
                    dsl, xsl = deltaB[:, :], dXB[:, :]
                    psel = p0164[0:64, ts(jj % 8, 128)]
                dA_t = pT.tile([128, LCF], bf16, name="dA_t")
                dxc = pT.tile([128, LCF], bf16, name="dxc")
                for h in range(2):
                    drep = mmtile(128, LC, "drep")
                    nc.tensor.matmul(drep[:], psel,
                                     dsl[:, bass.ds(h * LC, LC)],
                                     start=True, stop=True)
                    nc.scalar.activation(dA_t[:, ts(h, LC)], drep[:], AF.Exp,
                                         scale=acol[:, j:j + 1])
                    dxrep = mmtile(128, LC, "dxrep")
                    nc.tensor.matmul(dxrep[:], psel,
                                     xsl[:, bass.ds(h * LC, LC)],
                                     start=True, stop=True)
                    nc.scalar.activation(dxc[:, ts(h, LC)], dxrep[:], AF.Copy)
                dBu = pT.tile([128, LCF], bf16, name="dBu")
                nc.vector.tensor_tensor(dBu[:], dxc[:], brep[:], op=OP.mult)
                h_t = pT.tile([128, LCF], bf16, name="h_t")
                init = 0.0 if i == 0 else hlast[:, j:j + 1]
                nc.vector.tensor_tensor_scan(
                    h_t[:], dA_t[:], dBu[:], init, op0=OP.mult, op1=OP.add)
                nc.scalar.copy(hlast[:, j:j + 1], h_t[:, LCF - 1:LCF])
                ch = pT.tile([128, LCF], bf16, name="ch")
                nc.gpsimd.tensor_tensor(ch[:], h_t[:], crep[:], op=OP.mult)
                if j < 16:
                    nc.tensor.matmul(ypsA0[:], r01all[:, ts(jj, 128)],
                                     ch[:, 0:LC], start=(j == 0), stop=(j == 15))
                    nc.tensor.matmul(ypsA1[:], r01all[:, ts(jj, 128)],
                                     ch[:, LC:LCF], start=(j == 0), stop=(j == 15))
                else:
                    nc.tensor.matmul(ypsB0, r01ball[:, ts(jj, 64)],
                                     ch[:, 0:LC], start=(j == 16), stop=(j == 23),
                                     skip_group_check=True)
                    nc.tensor.matmul(ypsB1, r01ball[:, ts(jj, 64)],
                                     ch[:, LC:LCF], start=(j == 16), stop=(j == 23),
                                     skip_group_check=True)
            yA = pT.tile([DA, LCF], bf16, name="yA", bufs=1)
            yB = pT.tile([DB, LCF], bf16, name="yB", bufs=1)
            for h, (ya_ps, yb_ps) in enumerate([(ypsA0[:], ypsB0), (ypsA1[:], ypsB1)]):
                off = i * LCF + h * LC
                sl = bass.ds(off, LC)
                nc.vector.scalar_tensor_tensor(
                    yA[:, ts(h, LC)], xi2A[:, sl], dpA[:], ya_ps,
                    op0=OP.mult, op1=OP.add)
                nc.vector.scalar_tensor_tensor(
                    yB[:, ts(h, LC)], xi2B[:, sl], dpB[:], yb_ps,
                    op0=OP.mult, op1=OP.add)
            nc.vector.tensor_tensor(yA[:], yA[:], zsA[:, ts(i, LCF)], op=OP.mult)
            nc.vector.tensor_tensor(yB[:], yB[:], zsB[:, ts(i, LCF)], op=OP.mult)
            mout = pT.tile([C, LCF], bf16, name="mout", bufs=1)
            for h in range(2):
                mps = mmtile(C, LC, "mps")
                nc.tensor.matmul(mps[:], outwTA[:], yA[:, ts(h, LC)],
                                 start=True, stop=False)
                nc.tensor.matmul(mps[:], outwTB[:], yB[:, ts(h, LC)],
                                 start=False, stop=True)
                nc.any.tensor_copy(mout[:, ts(h, LC)], mps[:])
            # Z = xdq + mout^T computed wholly in PSUM: transpose matmul and
            # an identity matmul accumulate into one bank; DVE just copies out
            for r0 in range(0, 16, 4):
                zps = tptile(WQ, 4 * C, "zps")
                zps3 = zps.rearrange("p (r c) -> p r c", r=4)
                for k in range(4):
                    r = r0 + k
                    nc.tensor.matmul(zps3[:, k, :], mout[:, ts(r, WQ)],
                                     identb[:], start=True, stop=False)
                    nc.tensor.matmul(zps3[:, k, :], identb[0:WQ, 0:WQ],
                                     xdq3[:, :, 16 * i + r],
                                     start=False, stop=True)
                hq0 = 16 * i + r0
                nc.vector.tensor_copy(Z3[:, :, hq0:hq0 + 4],
                                      zps3.transpose([0, 2, 1]))
        pT.release()
        pG.release()

        # =============== Phase G: IDCT contribution ===============
        pH = tc.alloc_tile_pool(name="pH", bufs=1)
        # t7[hq, c, W] = sum_wq Z[wq, c, hq] * Mw_q[wq, W]  (per-c matmul)
        t7 = pH.tile([HQ, C * W], bf16)
        t7_3 = t7.rearrange("p (c w) -> p c w", c=C)
        for c0 in range(0, C, 4):
            t7ps = tptile(HQ, 4 * W, "t7ps")
            t7ps3 = t7ps.rearrange("p (c w) -> p c w", c=4)
            for k in range(4):
                nc.tensor.matmul(t7ps3[:, k, :], Z3[:, c0 + k, :],
                                 mwq64[64:128, :], start=True, stop=True)
            nc.scalar.activation(t7_3[:, c0:c0 + 4, :], t7ps3[:, :, :], AF.Copy)
        ctr_h = contrib
        pSo = tc.alloc_tile_pool(name="pSo", bufs=3)
        for i in range(24):
            cps = mmtile(H, LC, "cps")
            nc.tensor.matmul(cps[:], mhq[:], t7[:, ts(i, LC)], start=True, stop=True)
            csb = pSo.tile([H, LC], bf16, name="csb")
            nc.scalar.activation(csb[:], cps[:], AF.Copy)
            nc.sync.dma_start(
                ctr_h[:, ts(i, 4), :],
                csb[:].rearrange("h (c w) -> h c w", c=4))
        pSo.release()
        pH.release()
        pD_.release()
        ptp.release()
        ppy.release()
        pmm.release()
        consts.release()

    nc.compile()
    return nc


def _host_inputs(inputs):
    """Build the 8 per-core input maps."""
    x = inputs["x"]
    ln_w, ln_b = inputs["ln_w"], inputs["ln_b"]
    Mh = _dct_mat(H)
    Mw = _dct_mat(W)
    ident = np.eye(128, dtype=np.float32)
    # within-tile partition order p = 8*s + r (r = d offset, s = state):
    # makes row replication p -> p%8 a contiguous-partition doubling
    s01 = np.zeros((S, 128), np.float32)
    p0164 = np.zeros((128, 128 * 8), np.float32)
    r01all = np.zeros((128, 128 * 16), np.float32)
    r01ball = np.zeros((128, 64 * 8), np.float32)
    for p in range(128):
        s01[p // 8, p] = 1.0
        for j in range(8):
            p0164[8 * j + p % 8, 128 * j + p] = 1.0
            p0164[64 + 8 * j + p % 8, 128 * j + p] = 1.0
        for j in range(16):
            r01all[p, 128 * j + 8 * j + p % 8] = 1.0
        for j in range(8):
            r01ball[p, 64 * j + 8 * j + p % 8] = 1.0
    in_maps = []
    for k in range(8):
        b, q = k // 4, k % 4
        h0 = (q // 2) * HQ
        w0 = (q % 2) * WQ
        in_w2 = (inputs["in_w"][q] * ln_w[None, :]).astype(np.float32)
        bias_e = (inputs["in_w"][q] @ ln_b).astype(np.float32)
        A = (-np.exp(inputs["A_log"][q])).astype(np.float32)  # [D, S]
        acol = np.zeros((128, NT), np.float32)
        for j in range(NT):
            for p in range(128):
                acol[p, j] = A[j * 8 + p % 8, p // 8]
        m = {
            "xb": x[b],
            "xbh": np.ascontiguousarray(x[b].transpose(1, 0, 2)),
            "mhqT": Mh[h0:h0 + HQ, :].T,
            "mwqT": Mw[w0:w0 + WQ, :].T,
            "mhq": Mh[h0:h0 + HQ, :],
            "mwq": Mw[w0:w0 + WQ, :],
            "ident": ident,
            "inwT": in_w2.T,
            "biasi": bias_e[:D, None],
            "biasz": bias_e[D:, None],
            "convw": inputs["conv_w"][q],
            "convb": inputs["conv_b"][q][:, None],
            "xpwT": inputs["xp_w"][q].T,
            "dtwT": inputs["dt_w"][q].T,
            "dtb": inputs["dt_b"][q][:, None],
            "acol": acol,
            "dp": inputs["Dp"][q][:, None],
            "outwT": inputs["out_w"][q].T,
            "p0164": p0164,
            "s01": s01,
            "r01all": r01all,
            "r01ball": r01ball,
        }
        import ml_dtypes
        bf = ["inwT", "xpwT", "dtwT", "outwT", "p0164", "s01",
              "r01all", "r01ball", "xb", "xbh", "mhqT", "mwqT", "mhq", "mwq",
              "contrib"]
        in_maps.append({
            kk: np.ascontiguousarray(np.asarray(
                vv, ml_dtypes.bfloat16 if kk in bf else np.float32))
            for kk, vv in m.items()})
    return in_maps


def kernel(**inputs):
    from concourse import bass_utils
    inputs = {k: np.asarray(v) for k, v in inputs.items()}
    if "nc" not in _BUILT:
        _BUILT["nc"] = _build_nc()
    nc = _BUILT["nc"]
    in_maps = _host_inputs(inputs)
    res = bass_utils.run_bass_kernel_spmd(nc, in_maps, core_ids=list(range(8)))
    out = np.zeros((B, C, H, W), np.float32)
    for k in range(8):
        out[k // 4] += np.asarray(
            res.results[k]["contrib"], np.float32).transpose(1, 0, 2)
    return out


if __name__ == "__main__":
    # smoke: random inputs, shape check only
    rng = np.random.default_rng(0)
    demo = {
        "x": rng.standard_normal((B, C, H, W), np.float32),
        "ln_w": np.ones(C, np.float32), "ln_b": np.zeros(C, np.float32),
        "in_w": rng.standard_normal((4, 2 * D, C), np.float32) * 0.02,
        "conv_w": rng.standard_normal((4, D, KCONV), np.float32) * 0.02,
        "conv_b": np.zeros((4, D), np.float32),
        "xp_w": rng.standard_normal((4, RK + 2 * S, D), np.float32) * 0.02,
        "dt_w": rng.standard_normal((4, D, RK), np.float32) * 0.02,
        "dt_b": np.full((4, D), -4.0, np.float32),
        "A_log": np.tile(np.log(np.arange(1, S + 1, dtype=np.float32)), (4, D, 1)),
        "Dp": np.ones((4, D), np.float32),
        "out_w": rng.standard_normal((4, C, D), np.float32) * 0.02,
    }
    out = kernel(**demo)
    print("kernel output:", out.shape, out.dtype)



# revision 20
# speedup vs baseline: 1.0733x; 1.0733x over previous
"""Trainium2 Bass kernel for nn_EnhancedFreqLCBlock.

Self-contained: accepts FULL inputs, returns FULL output.
Sharding: 8 cores = 2 batches x 4 quadrant Mamba blocks (expert parallel).
Each core: mask -> quadrant 2D-DCT -> channel LN -> Mamba (hardware
tensor_tensor_scan recurrence) -> residual -> quadrant IDCT contribution.
Host sums the 4 quadrant contributions per batch.
"""
import numpy as np

B, C, H, W = 2, 96, 128, 128
HQ, WQ = H // 2, W // 2          # 64, 64
L = HQ * WQ                      # 4096
D = 192                          # d_inner
S = 16                           # d_state
RK = 6                           # dt_rank
KCONV = 4
NCHUNK = 8
LC = L // NCHUNK                 # 512
NT = (D * S) // 128              # 24 scan partition-tiles
DA, DB = 128, 64                 # d split 192 = 128 + 64

_BUILT = {}


def _dct_mat(N):
    n = np.arange(N)
    M = np.cos(np.pi * (2 * n[None, :] + 1) * n[:, None] / (2 * N)) * np.sqrt(2.0 / N)
    M[0] *= 1.0 / np.sqrt(2.0)
    return M.astype(np.float32)


def _build_nc():
    import concourse.bacc as bacc
    import concourse.bass as bass
    import concourse.mybir as mybir
    import concourse.tile as tile

    f32 = mybir.dt.float32
    bf16 = mybir.dt.bfloat16
    AF = mybir.ActivationFunctionType
    OP = mybir.AluOpType
    AX = mybir.AxisListType
    ts = bass.ts

    nc = bacc.Bacc()

    # ---------------- DRAM I/O ----------------
    xb = nc.dram_tensor("xb", [C, H, W], bf16, kind="ExternalInput")
    xbh = nc.dram_tensor("xbh", [H, C, W], bf16, kind="ExternalInput")
    d_mhqT = nc.dram_tensor("mhqT", [H, HQ], bf16, kind="ExternalInput")
    d_mwqT = nc.dram_tensor("mwqT", [W, WQ], bf16, kind="ExternalInput")
    d_mhq = nc.dram_tensor("mhq", [HQ, H], bf16, kind="ExternalInput")
    d_mwq = nc.dram_tensor("mwq", [WQ, W], bf16, kind="ExternalInput")
    d_ident = nc.dram_tensor("ident", [128, 128], f32, kind="ExternalInput")
    d_inwtap = nc.dram_tensor("inwtap", [C, KCONV * D], bf16,
                              kind="ExternalInput")
    d_inwz = nc.dram_tensor("inwz", [C, D], bf16, kind="ExternalInput")
    d_biasz = nc.dram_tensor("biasz", [D, 1], f32, kind="ExternalInput")
    d_bconv = nc.dram_tensor("bconv", [D, 1], f32, kind="ExternalInput")
    d_bc3 = nc.dram_tensor("bc3", [D, KCONV - 1], f32, kind="ExternalInput")
    d_xpwT = nc.dram_tensor("xpwT", [D, 80], bf16, kind="ExternalInput")
    d_dtwT = nc.dram_tensor("dtwT", [RK, D], bf16, kind="ExternalInput")
    d_dtb = nc.dram_tensor("dtb", [D, 1], f32, kind="ExternalInput")
    d_acol = nc.dram_tensor("acol", [128, NT], f32, kind="ExternalInput")
    d_dp = nc.dram_tensor("dp", [D, 1], f32, kind="ExternalInput")
    d_outwT = nc.dram_tensor("outwT", [D, C], bf16, kind="ExternalInput")
    d_p0164 = nc.dram_tensor("p0164", [128, 128 * 8], bf16, kind="ExternalInput")
    d_s01 = nc.dram_tensor("s01", [S, 128], bf16, kind="ExternalInput")
    d_r01all = nc.dram_tensor("r01all", [128, 128 * 16], bf16, kind="ExternalInput")
    d_r01ball = nc.dram_tensor("r01ball", [128, 64 * 8], bf16, kind="ExternalInput")
    contrib = nc.dram_tensor("contrib", [H, C, W], bf16, kind="ExternalOutput")
    # per-chunk DRAM scratch for the dX row-replication round-trip
    d_dxs = nc.dram_tensor("dxscratch", [NCHUNK // 2, D, 2 * LC], bf16,
                           kind="Internal")

    with tile.TileContext(nc) as tc:
        consts = tc.alloc_tile_pool(name="consts", bufs=1)
        # issue the big input loads before the ~30 const loads: SP
        # dispatches DMAs in program order and the mask path gates startup
        pD_ = tc.alloc_tile_pool(name="pD", bufs=1)
        pB = tc.alloc_tile_pool(name="pB", bufs=1)
        pXH = tc.alloc_tile_pool(name="pXH", bufs=1)
        pA = tc.alloc_tile_pool(name="pA", bufs=1)
        xc = pA.tile([C, H * W], bf16)
        xb_c = xb.rearrange("c h w -> c (h w)")
        for k in (4, 0, 1, 2, 3, 5, 6, 7):
            nc.sync.dma_start(xc[:, ts(k, 2048)], xb_c[:, ts(k, 2048)])
        xh = pXH.tile([H, C * W], bf16)
        xh3 = xh.rearrange("h (c w) -> h c w", c=C)
        # c-chunked loads keep full 128-partition spans (4x the DMA rate of
        # h-chunked loads)
        for i in range(4):
            nc.sync.dma_start(xh3[:, ts(i, 24), :], xbh[:, ts(i, 24), :])

        def cload(dram, shape, dt=f32):
            t = consts.tile(shape, dt, name=f"c_{dram.name}")
            nc.sync.dma_start(t[:], dram[:])
            return t

        def cload2(dram, dt=f32):
            ta = consts.tile([DA] + list(dram.shape[1:]), dt, name=f"cA_{dram.name}")
            nc.sync.dma_start(ta[:], dram[0:DA])
            tb = consts.tile([DB] + list(dram.shape[1:]), dt, name=f"cB_{dram.name}")
            nc.sync.dma_start(tb[:], dram[DA:D])
            return ta, tb

        mhqT = cload(d_mhqT, [H, HQ], bf16)
        mwqT = cload(d_mwqT, [W, WQ], bf16)
        mhq = cload(d_mhq, [HQ, H], bf16)
        mwq64 = consts.tile([128, W], bf16, name="c_mwq64")
        nc.sync.dma_start(mwq64[64:128, :], d_mwq[:])
        ident = cload(d_ident, [128, 128])
        identb = consts.tile([C, C], bf16, name="identb")
        nc.vector.tensor_copy(identb[:], ident[0:C, 0:C])
        inwtap = cload(d_inwtap, [C, KCONV * D], bf16)
        inwz = cload(d_inwz, [C, D], bf16)
        biaszA, biaszB = cload2(d_biasz)
        bconvA, bconvB = cload2(d_bconv)
        bc3A, bc3B = cload2(d_bc3)
        xpwTA, xpwTB = cload2(d_xpwT, bf16)
        dtwT = cload(d_dtwT, [RK, D], bf16)
        dtbA, dtbB = cload2(d_dtb)
        acol = cload(d_acol, [128, NT])
        dpA, dpB = cload2(d_dp)
        outwTA, outwTB = cload2(d_outwT, bf16)
        p0164 = cload(d_p0164, [128, 128 * 8], bf16)
        s01 = cload(d_s01, [S, 128], bf16)
        r01all = cload(d_r01all, [128, 128 * 16], bf16)
        r01ball = cload(d_r01ball, [128, 64 * 8], bf16)
        onesr = consts.tile([1, 128], f32)
        nc.vector.memset(onesr[:], 1.0)
        ones96b = consts.tile([C, 1], bf16)
        nc.vector.memset(ones96b[:], 1.0)
        eps64 = consts.tile([WQ, 1], f32)
        nc.vector.memset(eps64[:], 1e-5)

        # persistent psum pools (8 banks total: 4 + 2 + 2)
        pmm = tc.alloc_tile_pool(name="pmm", bufs=4, space="PSUM")
        ppy = tc.alloc_tile_pool(name="ppy", bufs=1, space="PSUM")
        ptp = tc.alloc_tile_pool(name="ptp", bufs=1, space="PSUM")

        def mmtile(p, n, nm):
            return pmm.tile([p, n], f32, name=nm, tag="mm")

        def tptile(p, n, nm, dt=f32):
            return ptp.tile([p, n], dt, name=nm, tag="tp")

        # =============== Phase A: mask ===============
        cpos = (H // 2) * W + (W // 2)
        center = xc[:, cpos:cpos + 1]                       # [96,1]
        cn_ps = tptile(1, 1, "cn_ps")
        nc.tensor.matmul(cn_ps[:], center, center, start=True, stop=True)
        s049 = pA.tile([1, 1], f32)
        nc.vector.tensor_scalar_mul(s049[:], cn_ps[:], 0.49)
        s049p = tptile(128, 1, "s049p")
        nc.tensor.matmul(s049p[:], onesr[:], s049[:], start=True, stop=True)
        s049b = pA.tile([128, 1], f32)
        nc.vector.tensor_copy(s049b[:], s049p[:])

        num_hw = pA.tile([128, 128], f32)
        ssq_hw = pA.tile([128, 128], f32)
        # num chunks pipeline through the 4-buffer mm psum pool; copies
        # alternate DVE/Scalar so neither engine throttles the matmul chain
        pSt = tc.alloc_tile_pool(name="pSt", bufs=4)
        for i in range(32):
            nps = mmtile(1, LC, "nps")
            nc.tensor.matmul(nps[:], center, xc[:, ts(i, LC)],
                             start=True, stop=True)
            nrow = pSt.tile([1, LC], f32, name="nrow")
            if i % 2 == 0:
                nc.vector.tensor_copy(nrow[:], nps[:])
            else:
                nc.scalar.copy(nrow[:], nps[:])
            nc.sync.dma_start(num_hw[ts(i, 4), :], nrow[:])
        pSt.release()
        # sum over c of x^2 computed in the [h, c, w] layout: square a
        # w-chunk on Scalar, reduce the (middle) c axis on DVE/GpSimd -> [h, w]
        pSq = tc.alloc_tile_pool(name="pSq", bufs=2)
        for i in range(4):
            sq = pSq.tile([128, C * 32], f32, name="sq")
            sq3 = sq.rearrange("h (c w) -> h c w", c=C)
            nc.scalar.activation(sq3[:, :, :], xh3[:, :, ts(i, 32)], AF.Square)
            nc.vector.tensor_reduce(
                ssq_hw[:, ts(i, 32)], sq3.transpose([0, 2, 1]), axis=AX.X,
                op=OP.add)
        pSq.release()

        thr = pA.tile([128, 128], f32)
        nc.scalar.activation(thr[:], ssq_hw[:], AF.Sqrt, bias=0.0, scale=s049b[:])
        nc.vector.tensor_scalar_add(thr[:], thr[:], 0.7e-6)
        mask_hw = pA.tile([128, 128], bf16)
        nc.vector.tensor_tensor(mask_hw[:], num_hw[:], thr[:], op=OP.is_ge)
        for i in range(4):
            nc.vector.tensor_tensor(
                 xh3[:, ts(i, 24), :], xh3[:, ts(i, 24), :],
                 mask_hw[:, None, :].broadcast_to([128, 24, 128]), op=OP.mult)
        pA.release()

        # =============== Phase B: forward DCT ===============
        # t2[w, c, hq] = sum_h x[h, c, w] * Mh_q[hq, h]  (per-c matmul, no
        # separate transpose pass)
        t2 = pB.tile([W, C * HQ], bf16)
        t2_3 = t2.rearrange("p (c q) -> p c q", c=C)
        for c0 in range(0, C, 8):
            tps = tptile(W, 8 * HQ, "tps")
            tps3 = tps.rearrange("p (c q) -> p c q", c=8)
            for k in range(8):
                nc.tensor.matmul(tps3[:, k, :], xh3[:, c0 + k, :], mhqT[:],
                                 start=True, stop=True)
            nc.scalar.activation(t2_3[:, c0:c0 + 8, :], tps3[:, :, :], AF.Copy)
        pXH.release()

        # xdqZ: rows 0:64 = xdq (base 0 for DVE pairing), rows 64:128 = Z
        xdqZ = pD_.tile([128, C * HQ], bf16)
        xdq3 = xdqZ.rearrange("p (c q) -> p c q", c=C)[0:HQ, :, :]
        Z3 = xdqZ.rearrange("p (c q) -> p c q", c=C)[HQ:128, :, :]
        xdq2 = xdqZ[0:HQ, :]
        Z2 = xdqZ[HQ:128, :]
        for i in range(12):
            xps = mmtile(WQ, LC, "xps")
            nc.tensor.matmul(xps[:], mwqT[:], t2[:, ts(i, LC)], start=True, stop=True)
            nc.any.tensor_copy(xdq2[:, ts(i, LC)], xps[:])
        pB.release()

        # =============== Phase C: LayerNorm over c ===============
        pG = tc.alloc_tile_pool(name="pG", bufs=1)
        pE = tc.alloc_tile_pool(name="pE", bufs=1)
        pC = tc.alloc_tile_pool(name="pC", bufs=1)
        # LN stats chunked to the 12 stage-2 copy chunks (8 c's each) so the
        # reduces overlap the DCT matmuls instead of serializing the machine
        smu = pC.tile([WQ, HQ], f32)
        ssq2 = pC.tile([WQ, HQ], f32)
        xn = pC.tile([WQ, C * HQ], bf16)  # first used as xdq^2 scratch
        xn3s = xn.rearrange("p (c q) -> p c q", c=C)
        pPart = tc.alloc_tile_pool(name="pPart", bufs=3)
        for cb in range(C // 8):
            csl = bass.ds(cb * 8, 8)
            nc.vector.tensor_tensor(xn3s[:, csl, :], xdq3[:, csl, :],
                                    xdq3[:, csl, :], op=OP.mult)
            pm = pPart.tile([WQ, HQ], f32, name="pm")
            nc.vector.tensor_reduce(
                pm[:], xdq3[:, csl, :].transpose([0, 2, 1]), axis=AX.X,
                op=OP.add)
            psq = pPart.tile([WQ, HQ], f32, name="psq")
            nc.vector.tensor_reduce(
                psq[:], xn3s[:, csl, :].transpose([0, 2, 1]), axis=AX.X,
                op=OP.add)
            if cb == 0:
                nc.vector.tensor_copy(smu[:], pm[:])
                nc.vector.tensor_copy(ssq2[:], psq[:])
            else:
                nc.vector.tensor_tensor(smu[:], smu[:], pm[:], op=OP.add)
                nc.vector.tensor_tensor(ssq2[:], ssq2[:], psq[:], op=OP.add)
        pPart.release()
        mu = pC.tile([WQ, HQ], f32)
        nc.vector.tensor_scalar_mul(mu[:], smu[:], 1.0 / C)
        var = pC.tile([WQ, HQ], f32)
        nc.vector.tensor_scalar_mul(ssq2[:], ssq2[:], 1.0 / C)
        nc.vector.tensor_tensor(var[:], mu[:], mu[:], op=OP.mult)
        nc.vector.tensor_tensor(var[:], ssq2[:], var[:], op=OP.subtract)
        sd = pC.tile([WQ, HQ], f32)
        nc.scalar.activation(sd[:], var[:], AF.Sqrt, bias=eps64[:])
        inv = pC.tile([WQ, HQ], f32)
        nc.vector.reciprocal(inv[:], sd[:])
        # bf16 stats so the normalize runs at DVE 2x rate; chunk by hq so
        # the transposes start before the whole tensor is normalized
        mub = pC.tile([WQ, HQ], bf16)
        nc.vector.tensor_copy(mub[:], mu[:])
        invb = pC.tile([WQ, HQ], bf16)
        nc.vector.tensor_copy(invb[:], inv[:])
        xn3 = xn.rearrange("p (c q) -> p c q", c=C)
        xn_c = pE.tile([C, L], bf16)
        xi2A = pG.tile([DA, L], bf16)
        zsA = pG.tile([DA, L], bf16)
        xi2B_t = pG.tile([DB, L], bf16, name="xi2B_t")
        zsB_t = pG.tile([DB, L], bf16, name="zsB_t")
        xi2B = xi2B_t[:, :]
        zsB = zsB_t[:, :]

        # ====== fused loop: LN-normalize -> in_proj(+conv taps) -> scan ======
        # (per 1024-col chunk so PE/Scalar front-end work overlaps the
        # DVE-bound scan of the previous chunk)
        pT = tc.alloc_tile_pool(name="pT", bufs=3)
        hlast = pG.tile([128, NT], bf16)
        LCF = 2 * LC
        for i in range(NCHUNK // 2):
            # --- LN normalize + transpose for hq rows [16i, 16i+16) ---
            h0 = 16 * i
            hsl = bass.ds(h0, 16)
            nc.vector.tensor_tensor(
                xn3[:, :, hsl], xdq3[:, :, hsl],
                mub[:, None, hsl].broadcast_to([WQ, C, 16]), op=OP.subtract)
            nc.vector.tensor_tensor(
                xn3[:, :, hsl], xn3[:, :, hsl],
                invb[:, None, hsl].broadcast_to([WQ, C, 16]), op=OP.mult)
            tps2 = tptile(C, 16 * WQ, "tps2", bf16)
            tps2_3 = tps2.rearrange("p (h q) -> p h q", h=16)
            for k in range(16):
                nc.tensor.matmul(tps2_3[:, k, :], xn3[:, :, h0 + k],
                                 identb[0:WQ, 0:WQ],
                                 is_transpose=True, start=True, stop=True)
            nc.scalar.activation(xn_c[:, h0 * WQ:(h0 + 16) * WQ], tps2[:], AF.Copy)

            # --- in_proj with the causal conv folded in as 4 shifted taps
            # accumulated in PSUM; silu applied straight from PSUM ---
            for c2 in range(2):
                c8 = 2 * i + c2
                t0 = c8 * LC
                psA = mmtile(128, LC, "psA")
                psBz = mmtile(128, LC, "psBz")
                psZ = mmtile(128, LC, "psZ")
                for k in range(KCONV):
                    if c8 == 0:
                        rhs = xn_c[:, 0:LC - k]
                        oA = psA[:, k:LC]
                        oB = psBz[0:64, k:LC]
                    else:
                        rhs = xn_c[:, t0 - k:t0 - k + LC]
                        oA = psA[:, :]
                        oB = psBz[0:64, :]
                    nc.tensor.matmul(oA, inwtap[:, k * D:k * D + DA], rhs,
                                     start=(k == 0), stop=(k == KCONV - 1))
                    nc.tensor.matmul(oB, inwtap[:, k * D + DA:(k + 1) * D], rhs,
                                     start=(k == 0), stop=(k == KCONV - 1),
                                     skip_group_check=True)
                nc.tensor.matmul(psZ[:], inwz[:, 0:DA], xn_c[:, ts(c8, LC)],
                                 start=True, stop=True)
                nc.tensor.matmul(psBz[64:128, :], inwz[:, DA:D],
                                 xn_c[:, ts(c8, LC)],
                                 start=True, stop=True, skip_group_check=True)
                nc.scalar.activation(xi2A[:, ts(c8, LC)], psA[:], AF.Silu,
                                     bias=bconvA[:])
                nc.scalar.activation(xi2B[:, ts(c8, LC)], psBz[0:64, :], AF.Silu,
                                     bias=bconvB[:])
                nc.scalar.activation(zsA[:, ts(c8, LC)], psZ[:], AF.Silu,
                                     bias=biaszA[:])
                nc.scalar.activation(zsB[:, ts(c8, LC)], psBz[64:128, :], AF.Silu,
                                     bias=biaszB[:])
                if c8 == 0:
                    # first KCONV-1 columns see a truncated tap sum -> redo
                    # their silu with the matching truncated bias
                    for t in range(KCONV - 1):
                        nc.scalar.activation(xi2A[:, t:t + 1], psA[:, t:t + 1],
                                             AF.Silu, bias=bc3A[:, t:t + 1])
                        nc.scalar.activation(xi2B[:, t:t + 1],
                                             psBz[0:64, t:t + 1],
                                             AF.Silu, bias=bc3B[:, t:t + 1])

            # --- xp projection (dt/B/C in one 80-row psum) ---
            dt_c = pT.tile([RK, LCF], bf16, name="dt_c", bufs=1)
            bm_c = pT.tile([S, LCF], bf16, name="bm_c", bufs=1)
            cm_c = pT.tile([S, LCF], bf16, name="cm_c", bufs=1)
            for h in range(2):
                off = i * LCF + h * LC
                sl = bass.ds(off, LC)
                ps80 = mmtile(80, LC, "ps80")
                nc.tensor.matmul(ps80[:], xpwTA[:], xi2A[:, sl],
                                 start=True, stop=False)
                nc.tensor.matmul(ps80[:], xpwTB[:], xi2B[:, sl],
                                 start=False, stop=True)
                nc.scalar.copy(dt_c[:, ts(h, LC)], ps80[0:RK, :])
                nc.scalar.copy(bm_c[:, ts(h, LC)], ps80[32:32 + S, :])
                nc.scalar.copy(cm_c[:, ts(h, LC)], ps80[64:64 + S, :])
            deltaA = pT.tile([DA, LCF], bf16, name="deltaA")
            deltaB = pT.tile([DB, LCF], bf16, name="deltaB")
            for h in range(2):
                dtpA = mmtile(DA, LC, "dtpA")
                nc.tensor.matmul(dtpA[:], dtwT[:, 0:DA], dt_c[0:RK, ts(h, LC)],
                                 start=True, stop=True)
                nc.scalar.activation(deltaA[:, ts(h, LC)], dtpA[:], AF.Exp,
                                     bias=dtbA[:])
                dtpB = mmtile(DB, LC, "dtpB")
                nc.tensor.matmul(dtpB[:], dtwT[:, DA:D], dt_c[0:RK, ts(h, LC)],
                                 start=True, stop=True)
                nc.scalar.activation(deltaB[:, ts(h, LC)], dtpB[:], AF.Exp,
                                     bias=dtbB[:])
            nc.scalar.activation(deltaA[:], deltaA[:], AF.Ln, bias=1.0)
            nc.scalar.activation(deltaB[:], deltaB[:], AF.Ln, bias=1.0)
            dXA = pT.tile([DA, LCF], bf16, name="dXA")
            nc.vector.tensor_tensor(dXA[:], deltaA[:],
                                    xi2A[:, ts(i, LCF)], op=OP.mult)
            dXB = pT.tile([DB, LCF], bf16, name="dXB")
            nc.vector.tensor_tensor(dXB[:], deltaB[:],
                                    xi2B[:, ts(i, LCF)], op=OP.mult)

            brep = pT.tile([128, LCF], bf16, name="brep")
            crep = pT.tile([128, LCF], bf16, name="crep")
            for h in range(2):
                brep_ps = mmtile(128, LC, "brep_ps")
                nc.tensor.matmul(brep_ps[:], s01[:], bm_c[:, ts(h, LC)],
                                 start=True, stop=True)
                nc.any.tensor_copy(brep[:, ts(h, LC)], brep_ps[:])
                crep_ps = mmtile(128, LC, "crep_ps")
                nc.tensor.matmul(crep_ps[:], s01[:], cm_c[:, ts(h, LC)],
                                 start=True, stop=True)
                nc.any.tensor_copy(crep[:, ts(h, LC)], crep_ps[:])

            ypsA0 = ppy.tile([128, LC], f32, name="ypsA0", tag="ypsA0")
            ypsA1 = ppy.tile([128, LC], f32, name="ypsA1", tag="ypsA1")
            ypsBp = ppy.tile([128, LC], f32, name="ypsBp", tag="ypsBp")
            ypsB0 = ypsBp[0:DB, :]
            ypsB1 = ypsBp[DB:128, :]
            for j in range(NT):
                jj = j if j < 16 else j - 16
                if j < 8:
                    dsl, xsl = deltaA[0:64, :], dXA[0:64, :]
                    psel = p0164[0:64, ts(jj % 8, 128)]
                elif j < 16:
                    dsl, xsl = deltaA[64:128, :], dXA[64:128, :]
                    psel = p0164[64:128, ts(jj % 8, 128)]
                else:
                    dsl, xsl = deltaB[:, :], dXB[:, :]
                    psel = p0164[0:64, ts(jj % 8, 128)]
                dA_t = pT.tile([128, LCF], bf16, name="dA_t")
                dxc = pT.tile([128, LCF], bf16, name="dxc")
                for h in range(2):
                    drep = mmtile(128, LC, "drep")
                    nc.tensor.matmul(drep[:], psel,
                                     dsl[:, bass.ds(h * LC, LC)],
                                     start=True, stop=True)
                    nc.scalar.activation(dA_t[:, ts(h, LC)], drep[:], AF.Exp,
                                         scale=acol[:, j:j + 1])
                    dxrep = mmtile(128, LC, "dxrep")
                    nc.tensor.matmul(dxrep[:], psel,
                                     xsl[:, bass.ds(h * LC, LC)],
                                     start=True, stop=True)
                    nc.scalar.activation(dxc[:, ts(h, LC)], dxrep[:], AF.Copy)
                dBu = pT.tile([128, LCF], bf16, name="dBu")
                nc.vector.tensor_tensor(dBu[:], dxc[:], brep[:], op=OP.mult)
                h_t = pT.tile([128, LCF], bf16, name="h_t")
                init = 0.0 if i == 0 else hlast[:, j:j + 1]
                nc.vector.tensor_tensor_scan(
                    h_t[:], dA_t[:], dBu[:], init, op0=OP.mult, op1=OP.add)
                nc.vector.tensor_copy(hlast[:, j:j + 1], h_t[:, LCF - 1:LCF])
                ch = pT.tile([128, LCF], bf16, name="ch")
                nc.vector.tensor_tensor(ch[:], h_t[:], crep[:], op=OP.mult)
                if j < 16:
                    nc.tensor.matmul(ypsA0[:], r01all[:, ts(jj, 128)],
                                     ch[:, 0:LC], start=(j == 0), stop=(j == 15))
                    nc.tensor.matmul(ypsA1[:], r01all[:, ts(jj, 128)],
                                     ch[:, LC:LCF], start=(j == 0), stop=(j == 15))
                else:
                    nc.tensor.matmul(ypsB0, r01ball[:, ts(jj, 64)],
                                     ch[:, 0:LC], start=(j == 16), stop=(j == 23),
                                     skip_group_check=True)
                    nc.tensor.matmul(ypsB1, r01ball[:, ts(jj, 64)],
                                     ch[:, LC:LCF], start=(j == 16), stop=(j == 23),
                                     skip_group_check=True)
            yA = pT.tile([DA, LCF], bf16, name="yA", bufs=1)
            yB = pT.tile([DB, LCF], bf16, name="yB", bufs=1)
            for h, (ya_ps, yb_ps) in enumerate([(ypsA0[:], ypsB0), (ypsA1[:], ypsB1)]):
                off = i * LCF + h * LC
                sl = bass.ds(off, LC)
                nc.vector.scalar_tensor_tensor(
                    yA[:, ts(h, LC)], xi2A[:, sl], dpA[:], ya_ps,
                    op0=OP.mult, op1=OP.add)
                nc.vector.scalar_tensor_tensor(
                    yB[:, ts(h, LC)], xi2B[:, sl], dpB[:], yb_ps,
                    op0=OP.mult, op1=OP.add)
            nc.vector.tensor_tensor(yA[:], yA[:], zsA[:, ts(i, LCF)], op=OP.mult)
            nc.vector.tensor_tensor(yB[:], yB[:], zsB[:, ts(i, LCF)], op=OP.mult)
            mout = pT.tile([C, LCF], bf16, name="mout", bufs=1)
            for h in range(2):
                mps = mmtile(C, LC, "mps")
                nc.tensor.matmul(mps[:], outwTA[:], yA[:, ts(h, LC)],
                                 start=True, stop=False)
                nc.tensor.matmul(mps[:], outwTB[:], yB[:, ts(h, LC)],
                                 start=False, stop=True)
                nc.any.tensor_copy(mout[:, ts(h, LC)], mps[:])
            # Z = xdq + mout^T computed wholly in PSUM: transpose matmul and
            # an identity matmul accumulate into one bank; DVE just copies out
            for r0 in range(0, 16, 4):
                zps = tptile(WQ, 4 * C, "zps")
                zps3 = zps.rearrange("p (r c) -> p r c", r=4)
                for k in range(4):
                    r = r0 + k
                    nc.tensor.matmul(zps3[:, k, :], mout[:, ts(r, WQ)],
                                     identb[:], start=True, stop=False)
                    nc.tensor.matmul(zps3[:, k, :], identb[0:WQ, 0:WQ],
                                     xdq3[:, :, 16 * i + r],
                                     start=False, stop=True)
                hq0 = 16 * i + r0
                nc.vector.tensor_copy(Z3[:, :, hq0:hq0 + 4],
                                      zps3.transpose([0, 2, 1]))
        pT.release()
        pC.release()
        pE.release()
        pG.release()

        # =============== Phase G: IDCT contribution ===============
        pH = tc.alloc_tile_pool(name="pH", bufs=1)
        # t7[hq, c, W] = sum_wq Z[wq, c, hq] * Mw_q[wq, W]  (per-c matmul)
        t7 = pH.tile([HQ, C * W], bf16)
        t7_3 = t7.rearrange("p (c w) -> p c w", c=C)
        for ci, c0 in enumerate(range(0, C, 4)):
            t7ps = tptile(HQ, 4 * W, "t7ps")
            t7ps3 = t7ps.rearrange("p (c w) -> p c w", c=4)
            for k in range(4):
                nc.tensor.matmul(t7ps3[:, k, :], Z3[:, c0 + k, :],
                                 mwq64[64:128, :], start=True, stop=True)
            if ci % 2 == 0:
                nc.scalar.activation(t7_3[:, c0:c0 + 4, :], t7ps3[:, :, :],
                                     AF.Copy)
            else:
                nc.vector.tensor_copy(t7_3[:, c0:c0 + 4, :], t7ps3[:, :, :])
        ctr_h = contrib
        pSo = tc.alloc_tile_pool(name="pSo", bufs=3)
        for i in range(24):
            cps = mmtile(H, LC, "cps")
            nc.tensor.matmul(cps[:], mhq[:], t7[:, ts(i, LC)], start=True, stop=True)
            csb = pSo.tile([H, LC], bf16, name="csb")
            if i % 2 == 0:
                nc.scalar.activation(csb[:], cps[:], AF.Copy)
            else:
                nc.vector.tensor_copy(csb[:], cps[:])
            nc.sync.dma_start(
                ctr_h[:, ts(i, 4), :],
                csb[:].rearrange("h (c w) -> h c w", c=4))
        pSo.release()
        pH.release()
        pD_.release()
        ptp.release()
        ppy.release()
        pmm.release()
        consts.release()

    nc.compile()
    return nc


def _host_inputs(inputs):
    """Build the 8 per-core input maps."""
    x = inputs["x"]
    ln_w, ln_b = inputs["ln_w"], inputs["ln_b"]
    Mh = _dct_mat(H)
    Mw = _dct_mat(W)
    ident = np.eye(128, dtype=np.float32)
    # within-tile partition order p = 8*s + r (r = d offset, s = state):
    # makes row replication p -> p%8 a contiguous-partition doubling
    s01 = np.zeros((S, 128), np.float32)
    p0164 = np.zeros((128, 128 * 8), np.float32)
    r01all = np.zeros((128, 128 * 16), np.float32)
    r01ball = np.zeros((128, 64 * 8), np.float32)
    for p in range(128):
        s01[p // 8, p] = 1.0
        for j in range(8):
            p0164[8 * j + p % 8, 128 * j + p] = 1.0
            p0164[64 + 8 * j + p % 8, 128 * j + p] = 1.0
        for j in range(16):
            r01all[p, 128 * j + 8 * j + p % 8] = 1.0
        for j in range(8):
            r01ball[p, 64 * j + 8 * j + p % 8] = 1.0
    in_maps = []
    for k in range(8):
        b, q = k // 4, k % 4
        h0 = (q // 2) * HQ
        w0 = (q % 2) * WQ
        in_w2 = (inputs["in_w"][q] * ln_w[None, :]).astype(np.float32)
        bias_e = (inputs["in_w"][q] @ ln_b).astype(np.float32)
        # conv folded into in_proj: tap k' applies weight conv_w[:, 3-k'] to
        # positions shifted back by k'
        convw = inputs["conv_w"][q].astype(np.float32)        # [D, 4]
        convb = inputs["conv_b"][q].astype(np.float32)        # [D]
        w_rev = convw[:, ::-1]
        WX = in_w2[:D]                                        # [D, C]
        inwtap = np.concatenate(
            [(WX * w_rev[:, kk][:, None]).T for kk in range(KCONV)], axis=1)
        bconv = bias_e[:D] * convw.sum(1) + convb
        bc3 = np.stack(
            [bias_e[:D] * w_rev[:, :t + 1].sum(1) + convb
             for t in range(KCONV - 1)], axis=1)              # [D, 3]
        xpw80 = np.zeros((D, 80), np.float32)
        xpwT = inputs["xp_w"][q].T
        xpw80[:, 0:RK] = xpwT[:, 0:RK]
        xpw80[:, 32:32 + S] = xpwT[:, RK:RK + S]
        xpw80[:, 64:64 + S] = xpwT[:, RK + S:RK + 2 * S]
        A = (-np.exp(inputs["A_log"][q])).astype(np.float32)  # [D, S]
        acol = np.zeros((128, NT), np.float32)
        for j in range(NT):
            for p in range(128):
                acol[p, j] = A[j * 8 + p % 8, p // 8]
        m = {
            "xb": x[b],
            "xbh": np.ascontiguousarray(x[b].transpose(1, 0, 2)),
            "mhqT": Mh[h0:h0 + HQ, :].T,
            "mwqT": Mw[w0:w0 + WQ, :].T,
            "mhq": Mh[h0:h0 + HQ, :],
            "mwq": Mw[w0:w0 + WQ, :],
            "ident": ident,
            "inwtap": inwtap,
            "inwz": in_w2[D:].T,
            "biasz": bias_e[D:, None],
            "bconv": bconv[:, None],
            "bc3": bc3,
            "xpwT": xpw80,
            "dtwT": inputs["dt_w"][q].T,
            "dtb": inputs["dt_b"][q][:, None],
            "acol": acol,
            "dp": inputs["Dp"][q][:, None],
            "outwT": inputs["out_w"][q].T,
            "p0164": p0164,
            "s01": s01,
            "r01all": r01all,
            "r01ball": r01ball,
        }
        import ml_dtypes
        bf = ["inwtap", "inwz", "xpwT", "dtwT", "outwT", "p0164", "s01",
              "r01all", "r01ball", "xb", "xbh", "mhqT", "mwqT", "mhq", "mwq",
              "contrib"]
        in_maps.append({
            kk: np.ascontiguousarray(np.asarray(
                vv, ml_dtypes.bfloat16 if kk in bf else np.float32))
            for kk, vv in m.items()})
    return in_maps


def kernel(**inputs):
    from concourse import bass_utils
    inputs = {k: np.asarray(v) for k, v in inputs.items()}
    if "nc" not in _BUILT:
        _BUILT["nc"] = _build_nc()
    nc = _BUILT["nc"]
    in_maps = _host_inputs(inputs)
    res = bass_utils.run_bass_kernel_spmd(nc, in_maps, core_ids=list(range(8)))
    out = np.zeros((B, C, H, W), np.float32)
    for k in range(8):
        out[k // 4] += np.asarray(
            res.results[k]["contrib"], np.float32).transpose(1, 0, 2)
    return out


if __name__ == "__main__":
    # smoke: random inputs, shape check only
    rng = np.random.default_rng(0)
    demo = {
        "x": rng.standard_normal((B, C, H, W), np.float32),
        "ln_w": np.ones(C, np.float32), "ln_b": np.zeros(C, np.float32),
        "in_w": rng.standard_normal((4, 2 * D, C), np.float32) * 0.02,
        "conv_w": rng.standard_normal((4, D, KCONV), np.float32) * 0.02,
        "conv_b": np.zeros((4, D), np.float32),
        "xp_w": rng.standard_normal((4, RK + 2 * S, D), np.float32) * 0.02,
        "dt_w": rng.standard_normal((4, D, RK), np.float32) * 0.02,
        "dt_b": np.full((4, D), -4.0, np.float32),
        "A_log": np.tile(np.log(np.arange(1, S + 1, dtype=np.float32)), (4, D, 1)),
        "Dp": np.ones((4, D), np.float32),
        "out_w": rng.standard_normal((4, C, D), np.float32) * 0.02,
    }
    out = kernel(**demo)
    print("kernel output:", out.shape, out.dtype)



# revision 27
# speedup vs baseline: 1.1129x; 1.0369x over previous
"""Trainium2 Bass kernel for nn_EnhancedFreqLCBlock.

Self-contained: accepts FULL inputs, returns FULL output.
Sharding: 8 cores = 2 batches x 4 quadrant Mamba blocks (expert parallel).
Each core: mask -> quadrant 2D-DCT -> channel LN -> Mamba (hardware
tensor_tensor_scan recurrence) -> residual -> quadrant IDCT contribution.
Host sums the 4 quadrant contributions per batch.
"""
import numpy as np

B, C, H, W = 2, 96, 128, 128
HQ, WQ = H // 2, W // 2          # 64, 64
L = HQ * WQ                      # 4096
D = 192                          # d_inner
S = 16                           # d_state
RK = 6                           # dt_rank
KCONV = 4
NCHUNK = 8
LC = L // NCHUNK                 # 512
NT = (D * S) // 128              # 24 scan partition-tiles
DA, DB = 128, 64                 # d split 192 = 128 + 64

_BUILT = {}


def _dct_mat(N):
    n = np.arange(N)
    M = np.cos(np.pi * (2 * n[None, :] + 1) * n[:, None] / (2 * N)) * np.sqrt(2.0 / N)
    M[0] *= 1.0 / np.sqrt(2.0)
    return M.astype(np.float32)


def _build_nc():
    import concourse.bacc as bacc
    import concourse.bass as bass
    import concourse.mybir as mybir
    import concourse.tile as tile

    f32 = mybir.dt.float32
    bf16 = mybir.dt.bfloat16
    AF = mybir.ActivationFunctionType
    OP = mybir.AluOpType
    AX = mybir.AxisListType
    ts = bass.ts

    nc = bacc.Bacc()

    # ---------------- DRAM I/O ----------------
    xb = nc.dram_tensor("xb", [C, H, W], bf16, kind="ExternalInput")
    xbh = nc.dram_tensor("xbh", [H, C, W], bf16, kind="ExternalInput")
    d_mhqT = nc.dram_tensor("mhqT", [H, HQ], bf16, kind="ExternalInput")
    d_mwqT = nc.dram_tensor("mwqT", [W, WQ], bf16, kind="ExternalInput")
    d_mhq = nc.dram_tensor("mhq", [HQ, H], bf16, kind="ExternalInput")
    d_mwq = nc.dram_tensor("mwq", [WQ, W], bf16, kind="ExternalInput")
    d_ident = nc.dram_tensor("ident", [128, 128], f32, kind="ExternalInput")
    d_inwtap = nc.dram_tensor("inwtap", [C, KCONV * D], bf16,
                              kind="ExternalInput")
    d_inwz = nc.dram_tensor("inwz", [C, D], bf16, kind="ExternalInput")
    d_biasz = nc.dram_tensor("biasz", [D, 1], f32, kind="ExternalInput")
    d_bconv = nc.dram_tensor("bconv", [D, 1], f32, kind="ExternalInput")
    d_bc3 = nc.dram_tensor("bc3", [D, KCONV - 1], f32, kind="ExternalInput")
    d_xpwT = nc.dram_tensor("xpwT", [D, 80], bf16, kind="ExternalInput")
    d_dtwT = nc.dram_tensor("dtwT", [RK, D], bf16, kind="ExternalInput")
    d_dtb = nc.dram_tensor("dtb", [D, 1], f32, kind="ExternalInput")
    d_acol = nc.dram_tensor("acol", [128, NT], f32, kind="ExternalInput")
    d_dpdA = nc.dram_tensor("dpdA", [DA, DA], bf16, kind="ExternalInput")
    d_dpdB = nc.dram_tensor("dpdB", [DB, DB], bf16, kind="ExternalInput")
    d_outwT = nc.dram_tensor("outwT", [D, C], bf16, kind="ExternalInput")
    d_p0164 = nc.dram_tensor("p0164", [128, 128 * 8], bf16, kind="ExternalInput")
    d_s01 = nc.dram_tensor("s01", [S, 128], bf16, kind="ExternalInput")
    d_r01all = nc.dram_tensor("r01all", [128, 128 * 16], bf16, kind="ExternalInput")
    d_r01ball = nc.dram_tensor("r01ball", [128, 64 * 8], bf16, kind="ExternalInput")
    contrib = nc.dram_tensor("contrib", [H, C, W], bf16, kind="ExternalOutput")
    # per-chunk DRAM scratch for the dX row-replication round-trip
    d_dxs = nc.dram_tensor("dxscratch", [NCHUNK // 2, D, 2 * LC], bf16,
                           kind="Internal")

    with tile.TileContext(nc) as tc:
        consts = tc.alloc_tile_pool(name="consts", bufs=1)
        # issue the big input loads before the ~30 const loads: SP
        # dispatches DMAs in program order and the mask path gates startup
        pD_ = tc.alloc_tile_pool(name="pD", bufs=1)
        pB = tc.alloc_tile_pool(name="pB", bufs=1)
        pXH = tc.alloc_tile_pool(name="pXH", bufs=1)
        pA = tc.alloc_tile_pool(name="pA", bufs=1)
        xc = pA.tile([C, H * W], bf16)
        xb_c = xb.rearrange("c h w -> c (h w)")
        for k in (4, 0, 1, 2, 3, 5, 6, 7):
            nc.sync.dma_start(xc[:, ts(k, 2048)], xb_c[:, ts(k, 2048)])
        xh = pXH.tile([H, C * W], bf16)
        xh3 = xh.rearrange("h (c w) -> h c w", c=C)
        # c-chunked loads keep full 128-partition spans (4x the DMA rate of
        # h-chunked loads)
        for i in range(4):
            nc.sync.dma_start(xh3[:, ts(i, 24), :], xbh[:, ts(i, 24), :])

        def cload(dram, shape, dt=f32):
            t = consts.tile(shape, dt, name=f"c_{dram.name}")
            nc.sync.dma_start(t[:], dram[:])
            return t

        def cload2(dram, dt=f32):
            ta = consts.tile([DA] + list(dram.shape[1:]), dt, name=f"cA_{dram.name}")
            nc.sync.dma_start(ta[:], dram[0:DA])
            tb = consts.tile([DB] + list(dram.shape[1:]), dt, name=f"cB_{dram.name}")
            nc.sync.dma_start(tb[:], dram[DA:D])
            return ta, tb

        mhqT = cload(d_mhqT, [H, HQ], bf16)
        mwqT = cload(d_mwqT, [W, WQ], bf16)
        mhq = cload(d_mhq, [HQ, H], bf16)
        mwq64 = consts.tile([128, W], bf16, name="c_mwq64")
        nc.sync.dma_start(mwq64[64:128, :], d_mwq[:])
        ident = cload(d_ident, [128, 128])
        identb = consts.tile([C, C], bf16, name="identb")
        nc.vector.tensor_copy(identb[:], ident[0:C, 0:C])
        inwtap = cload(d_inwtap, [C, KCONV * D], bf16)
        inwz = cload(d_inwz, [C, D], bf16)
        biaszA, biaszB = cload2(d_biasz)
        bconvA, bconvB = cload2(d_bconv)
        bc3A, bc3B = cload2(d_bc3)
        xpwTA, xpwTB = cload2(d_xpwT, bf16)
        dtwT = cload(d_dtwT, [RK, D], bf16)
        dtbA, dtbB = cload2(d_dtb)
        acol = cload(d_acol, [128, NT])
        dpdA = cload(d_dpdA, [DA, DA], bf16)
        dpdB = cload(d_dpdB, [DB, DB], bf16)
        outwTA, outwTB = cload2(d_outwT, bf16)
        p0164 = cload(d_p0164, [128, 128 * 8], bf16)
        s01 = cload(d_s01, [S, 128], bf16)
        r01all = cload(d_r01all, [128, 128 * 16], bf16)
        r01ball = cload(d_r01ball, [128, 64 * 8], bf16)
        onesr = consts.tile([1, 128], f32)
        nc.vector.memset(onesr[:], 1.0)
        ones96b = consts.tile([C, 1], bf16)
        nc.vector.memset(ones96b[:], 1.0)
        eps64 = consts.tile([WQ, 1], f32)
        nc.vector.memset(eps64[:], 1e-5)

        # persistent psum pools (8 banks total: 4 + 2 + 2)
        pmm = tc.alloc_tile_pool(name="pmm", bufs=4, space="PSUM")
        ppy = tc.alloc_tile_pool(name="ppy", bufs=1, space="PSUM")
        ptp = tc.alloc_tile_pool(name="ptp", bufs=1, space="PSUM")

        def mmtile(p, n, nm):
            return pmm.tile([p, n], f32, name=nm, tag="mm")

        def tptile(p, n, nm, dt=f32):
            return ptp.tile([p, n], dt, name=nm, tag="tp")

        # =============== Phase A: mask ===============
        cpos = (H // 2) * W + (W // 2)
        center = xc[:, cpos:cpos + 1]                       # [96,1]
        cn_ps = tptile(1, 1, "cn_ps")
        nc.tensor.matmul(cn_ps[:], center, center, start=True, stop=True)
        s049 = pA.tile([1, 1], f32)
        nc.vector.tensor_scalar_mul(s049[:], cn_ps[:], 0.49)
        s049p = tptile(128, 1, "s049p")
        nc.tensor.matmul(s049p[:], onesr[:], s049[:], start=True, stop=True)
        s049b = pA.tile([128, 1], f32)
        nc.vector.tensor_copy(s049b[:], s049p[:])

        num_hw = pA.tile([128, 128], f32)
        ssq_hw = pA.tile([128, 128], f32)
        # num chunks pipeline through the 4-buffer mm psum pool; copies
        # alternate DVE/Scalar so neither engine throttles the matmul chain
        pSt = tc.alloc_tile_pool(name="pSt", bufs=4)
        for i in range(32):
            nps = mmtile(1, LC, "nps")
            nc.tensor.matmul(nps[:], center, xc[:, ts(i, LC)],
                             start=True, stop=True)
            nrow = pSt.tile([1, LC], f32, name="nrow")
            if i % 2 == 0:
                nc.vector.tensor_copy(nrow[:], nps[:])
            else:
                nc.scalar.copy(nrow[:], nps[:])
            nc.sync.dma_start(num_hw[ts(i, 4), :], nrow[:])
        pSt.release()
        # sum over c of x^2 computed in the [h, c, w] layout: square a
        # w-chunk on Scalar, reduce the (middle) c axis on DVE/GpSimd -> [h, w]
        pSq = tc.alloc_tile_pool(name="pSq", bufs=2)
        for i in range(4):
            sq = pSq.tile([128, C * 32], f32, name="sq")
            sq3 = sq.rearrange("h (c w) -> h c w", c=C)
            nc.scalar.activation(sq3[:, :, :], xh3[:, :, ts(i, 32)], AF.Square)
            nc.vector.tensor_reduce(
                ssq_hw[:, ts(i, 32)], sq3.transpose([0, 2, 1]), axis=AX.X,
                op=OP.add)
        pSq.release()

        thr = pA.tile([128, 128], f32)
        nc.scalar.activation(thr[:], ssq_hw[:], AF.Sqrt, bias=0.0, scale=s049b[:])
        nc.vector.tensor_scalar_add(thr[:], thr[:], 0.7e-6)
        mask_hw = pA.tile([128, 128], bf16)
        nc.vector.tensor_tensor(mask_hw[:], num_hw[:], thr[:], op=OP.is_ge)
        for i in range(4):
            nc.vector.tensor_tensor(
                 xh3[:, ts(i, 24), :], xh3[:, ts(i, 24), :],
                 mask_hw[:, None, :].broadcast_to([128, 24, 128]), op=OP.mult)
        pA.release()

        # =============== Phase B: forward DCT ===============
        # t2[w, c, hq] = sum_h x[h, c, w] * Mh_q[hq, h]  (per-c matmul, no
        # separate transpose pass)
        t2 = pB.tile([W, C * HQ], bf16)
        t2_3 = t2.rearrange("p (c q) -> p c q", c=C)
        for c0 in range(0, C, 8):
            tps = tptile(W, 8 * HQ, "tps")
            tps3 = tps.rearrange("p (c q) -> p c q", c=8)
            for k in range(8):
                nc.tensor.matmul(tps3[:, k, :], xh3[:, c0 + k, :], mhqT[:],
                                 start=True, stop=True)
            nc.scalar.activation(t2_3[:, c0:c0 + 8, :], tps3[:, :, :], AF.Copy)
        pXH.release()

        # xdqZ: rows 0:64 = xdq (base 0 for DVE pairing), rows 64:128 = Z
        xdqZ = pD_.tile([128, C * HQ], bf16)
        xdq3 = xdqZ.rearrange("p (c q) -> p c q", c=C)[0:HQ, :, :]
        Z3 = xdqZ.rearrange("p (c q) -> p c q", c=C)[HQ:128, :, :]
        xdq2 = xdqZ[0:HQ, :]
        Z2 = xdqZ[HQ:128, :]
        for i in range(12):
            xps = mmtile(WQ, LC, "xps")
            nc.tensor.matmul(xps[:], mwqT[:], t2[:, ts(i, LC)], start=True, stop=True)
            nc.any.tensor_copy(xdq2[:, ts(i, LC)], xps[:])
        pB.release()

        # =============== Phase C: LayerNorm over c ===============
        pG = tc.alloc_tile_pool(name="pG", bufs=1)
        pE = tc.alloc_tile_pool(name="pE", bufs=1)
        pC = tc.alloc_tile_pool(name="pC", bufs=1)
        # LN stats chunked to the 12 stage-2 copy chunks (8 c's each) so the
        # reduces overlap the DCT matmuls instead of serializing the machine
        smu = pC.tile([WQ, HQ], f32)
        ssq2 = pC.tile([WQ, HQ], f32)
        xn = pC.tile([WQ, C * HQ], bf16)  # first used as xdq^2 scratch
        xn3s = xn.rearrange("p (c q) -> p c q", c=C)
        pPart = tc.alloc_tile_pool(name="pPart", bufs=3)
        for cb in range(C // 8):
            csl = bass.ds(cb * 8, 8)
            nc.vector.tensor_tensor(xn3s[:, csl, :], xdq3[:, csl, :],
                                    xdq3[:, csl, :], op=OP.mult)
            pm = pPart.tile([WQ, HQ], f32, name="pm")
            nc.vector.tensor_reduce(
                pm[:], xdq3[:, csl, :].transpose([0, 2, 1]), axis=AX.X,
                op=OP.add)
            psq = pPart.tile([WQ, HQ], f32, name="psq")
            nc.vector.tensor_reduce(
                psq[:], xn3s[:, csl, :].transpose([0, 2, 1]), axis=AX.X,
                op=OP.add)
            if cb == 0:
                nc.vector.tensor_copy(smu[:], pm[:])
                nc.vector.tensor_copy(ssq2[:], psq[:])
            else:
                nc.vector.tensor_tensor(smu[:], smu[:], pm[:], op=OP.add)
                nc.vector.tensor_tensor(ssq2[:], ssq2[:], psq[:], op=OP.add)
        pPart.release()
        mu = pC.tile([WQ, HQ], f32)
        nc.vector.tensor_scalar_mul(mu[:], smu[:], 1.0 / C)
        var = pC.tile([WQ, HQ], f32)
        nc.vector.tensor_scalar_mul(ssq2[:], ssq2[:], 1.0 / C)
        nc.vector.tensor_tensor(var[:], mu[:], mu[:], op=OP.mult)
        nc.vector.tensor_tensor(var[:], ssq2[:], var[:], op=OP.subtract)
        sd = pC.tile([WQ, HQ], f32)
        nc.scalar.activation(sd[:], var[:], AF.Sqrt, bias=eps64[:])
        inv = pC.tile([WQ, HQ], f32)
        nc.vector.reciprocal(inv[:], sd[:])
        # bf16 stats so the normalize runs at DVE 2x rate; chunk by hq so
        # the transposes start before the whole tensor is normalized
        mub = pC.tile([WQ, HQ], bf16)
        nc.vector.tensor_copy(mub[:], mu[:])
        invb = pC.tile([WQ, HQ], bf16)
        nc.vector.tensor_copy(invb[:], inv[:])
        xn3 = xn.rearrange("p (c q) -> p c q", c=C)
        xn_c = pE.tile([C, L], bf16)
        xi2A = pG.tile([DA, L], bf16)
        zsA = pG.tile([DA, L], bf16)
        xi2B_t = pG.tile([DB, L], bf16, name="xi2B_t")
        zsB_t = pG.tile([DB, L], bf16, name="zsB_t")
        xi2B = xi2B_t[:, :]
        zsB = zsB_t[:, :]

        # ====== fused loop: LN-normalize -> in_proj(+conv taps) -> scan ======
        # (per 1024-col chunk so PE/Scalar front-end work overlaps the
        # DVE-bound scan of the previous chunk)
        pT = tc.alloc_tile_pool(name="pT", bufs=3)
        hlast = pG.tile([128, NT], bf16)
        LCF = 2 * LC
        for i in range(NCHUNK // 2):
            # --- LN normalize + transpose for hq rows [16i, 16i+16) ---
            h0 = 16 * i
            hsl = bass.ds(h0, 16)
            nc.vector.tensor_tensor(
                xn3[:, :, hsl], xdq3[:, :, hsl],
                mub[:, None, hsl].broadcast_to([WQ, C, 16]), op=OP.subtract)
            nc.vector.tensor_tensor(
                xn3[:, :, hsl], xn3[:, :, hsl],
                invb[:, None, hsl].broadcast_to([WQ, C, 16]), op=OP.mult)
            tps2 = tptile(C, 16 * WQ, "tps2", bf16)
            tps2_3 = tps2.rearrange("p (h q) -> p h q", h=16)
            for k in range(16):
                nc.tensor.matmul(tps2_3[:, k, :], xn3[:, :, h0 + k],
                                 identb[0:WQ, 0:WQ],
                                 is_transpose=True, start=True, stop=True)
            nc.scalar.activation(xn_c[:, h0 * WQ:(h0 + 16) * WQ], tps2[:], AF.Copy)

            # --- in_proj with the causal conv folded in as 4 shifted taps
            # accumulated in PSUM; silu applied straight from PSUM ---
            for c2 in range(2):
                c8 = 2 * i + c2
                t0 = c8 * LC
                psA = mmtile(128, LC, "psA")
                psBz = mmtile(128, LC, "psBz")
                psZ = mmtile(128, LC, "psZ")
                for k in range(KCONV):
                    if c8 == 0:
                        rhs = xn_c[:, 0:LC - k]
                        oA = psA[:, k:LC]
                        oB = psBz[0:64, k:LC]
                    else:
                        rhs = xn_c[:, t0 - k:t0 - k + LC]
                        oA = psA[:, :]
                        oB = psBz[0:64, :]
                    nc.tensor.matmul(oA, inwtap[:, k * D:k * D + DA], rhs,
                                     start=(k == 0), stop=(k == KCONV - 1))
                    nc.tensor.matmul(oB, inwtap[:, k * D + DA:(k + 1) * D], rhs,
                                     start=(k == 0), stop=(k == KCONV - 1),
                                     skip_group_check=True)
                nc.tensor.matmul(psZ[:], inwz[:, 0:DA], xn_c[:, ts(c8, LC)],
                                 start=True, stop=True)
                nc.tensor.matmul(psBz[64:128, :], inwz[:, DA:D],
                                 xn_c[:, ts(c8, LC)],
                                 start=True, stop=True, skip_group_check=True)
                nc.scalar.activation(xi2A[:, ts(c8, LC)], psA[:], AF.Silu,
                                     bias=bconvA[:])
                nc.scalar.activation(xi2B[:, ts(c8, LC)], psBz[0:64, :], AF.Silu,
                                     bias=bconvB[:])
                nc.scalar.activation(zsA[:, ts(c8, LC)], psZ[:], AF.Silu,
                                     bias=biaszA[:])
                nc.scalar.activation(zsB[:, ts(c8, LC)], psBz[64:128, :], AF.Silu,
                                     bias=biaszB[:])
                if c8 == 0:
                    # first KCONV-1 columns see a truncated tap sum -> redo
                    # their silu with the matching truncated bias
                    for t in range(KCONV - 1):
                        nc.scalar.activation(xi2A[:, t:t + 1], psA[:, t:t + 1],
                                             AF.Silu, bias=bc3A[:, t:t + 1])
                        nc.scalar.activation(xi2B[:, t:t + 1],
                                             psBz[0:64, t:t + 1],
                                             AF.Silu, bias=bc3B[:, t:t + 1])

            # --- xp projection (dt/B/C in one 80-row psum) ---
            dt_c = pT.tile([RK, LCF], bf16, name="dt_c", bufs=1)
            bm_c = pT.tile([S, LCF], bf16, name="bm_c", bufs=1)
            cm_c = pT.tile([S, LCF], bf16, name="cm_c", bufs=1)
            for h in range(2):
                off = i * LCF + h * LC
                sl = bass.ds(off, LC)
                ps80 = mmtile(80, LC, "ps80")
                nc.tensor.matmul(ps80[:], xpwTA[:], xi2A[:, sl],
                                 start=True, stop=False)
                nc.tensor.matmul(ps80[:], xpwTB[:], xi2B[:, sl],
                                 start=False, stop=True)
                nc.scalar.copy(dt_c[:, ts(h, LC)], ps80[0:RK, :])
                nc.scalar.copy(bm_c[:, ts(h, LC)], ps80[32:32 + S, :])
                nc.scalar.copy(cm_c[:, ts(h, LC)], ps80[64:64 + S, :])
            deltaA = pT.tile([DA, LCF], bf16, name="deltaA")
            deltaB = pT.tile([DB, LCF], bf16, name="deltaB")
            for h in range(2):
                dtpA = mmtile(DA, LC, "dtpA")
                nc.tensor.matmul(dtpA[:], dtwT[:, 0:DA], dt_c[0:RK, ts(h, LC)],
                                 start=True, stop=True)
                nc.scalar.activation(deltaA[:, ts(h, LC)], dtpA[:], AF.Exp,
                                     bias=dtbA[:])
                dtpB = mmtile(DB, LC, "dtpB")
                nc.tensor.matmul(dtpB[:], dtwT[:, DA:D], dt_c[0:RK, ts(h, LC)],
                                 start=True, stop=True)
                nc.scalar.activation(deltaB[:, ts(h, LC)], dtpB[:], AF.Exp,
                                     bias=dtbB[:])
            nc.scalar.activation(deltaA[:], deltaA[:], AF.Ln, bias=1.0)
            nc.scalar.activation(deltaB[:], deltaB[:], AF.Ln, bias=1.0)
            dXA = pT.tile([DA, LCF], bf16, name="dXA")
            nc.vector.tensor_tensor(dXA[:], deltaA[:],
                                    xi2A[:, ts(i, LCF)], op=OP.mult)
            dXB = pT.tile([DB, LCF], bf16, name="dXB")
            nc.vector.tensor_tensor(dXB[:], deltaB[:],
                                    xi2B[:, ts(i, LCF)], op=OP.mult)

            brep = pT.tile([128, LCF], bf16, name="brep")
            crep = pT.tile([128, LCF], bf16, name="crep")
            for h in range(2):
                brep_ps = mmtile(128, LC, "brep_ps")
                nc.tensor.matmul(brep_ps[:], s01[:], bm_c[:, ts(h, LC)],
                                 start=True, stop=True)
                nc.any.tensor_copy(brep[:, ts(h, LC)], brep_ps[:])
                crep_ps = mmtile(128, LC, "crep_ps")
                nc.tensor.matmul(crep_ps[:], s01[:], cm_c[:, ts(h, LC)],
                                 start=True, stop=True)
                nc.any.tensor_copy(crep[:, ts(h, LC)], crep_ps[:])

            ypsA0 = ppy.tile([128, LC], f32, name="ypsA0", tag="ypsA0")
            ypsA1 = ppy.tile([128, LC], f32, name="ypsA1", tag="ypsA1")
            ypsBp = ppy.tile([128, LC], f32, name="ypsBp", tag="ypsBp")
            ypsB0 = ypsBp[0:DB, :]
            ypsB1 = ypsBp[DB:128, :]
            for j in range(NT):
                jj = j if j < 16 else j - 16
                if j < 8:
                    dsl, xsl = deltaA[0:64, :], dXA[0:64, :]
                    psel = p0164[0:64, ts(jj % 8, 128)]
                elif j < 16:
                    dsl, xsl = deltaA[64:128, :], dXA[64:128, :]
                    psel = p0164[64:128, ts(jj % 8, 128)]
                else:
                    dsl, xsl = deltaB[:, :], dXB[:, :]
                    psel = p0164[0:64, ts(jj % 8, 128)]
                dA_t = pT.tile([128, LCF], bf16, name="dA_t")
                dxc = pT.tile([128, LCF], bf16, name="dxc")
                for h in range(2):
                    drep = mmtile(128, LC, "drep")
                    nc.tensor.matmul(drep[:], psel,
                                     dsl[:, bass.ds(h * LC, LC)],
                                     start=True, stop=True)
                    nc.scalar.activation(dA_t[:, ts(h, LC)], drep[:], AF.Exp,
                                         scale=acol[:, j:j + 1])
                    dxrep = mmtile(128, LC, "dxrep")
                    nc.tensor.matmul(dxrep[:], psel,
                                     xsl[:, bass.ds(h * LC, LC)],
                                     start=True, stop=True)
                    nc.scalar.activation(dxc[:, ts(h, LC)], dxrep[:], AF.Copy)
                dBu = pT.tile([128, LCF], bf16, name="dBu")
                nc.vector.tensor_tensor(dBu[:], dxc[:], brep[:], op=OP.mult)
                h_t = pT.tile([128, LCF], bf16, name="h_t")
                init = 0.0 if i == 0 else hlast[:, j:j + 1]
                nc.vector.tensor_tensor_scan(
                    h_t[:], dA_t[:], dBu[:], init, op0=OP.mult, op1=OP.add)
                nc.vector.tensor_copy(hlast[:, j:j + 1], h_t[:, LCF - 1:LCF])
                ch = pT.tile([128, LCF], bf16, name="ch")
                nc.vector.tensor_tensor(ch[:], h_t[:], crep[:], op=OP.mult)
                if j < 16:
                    nc.tensor.matmul(ypsA0[:], r01all[:, ts(jj, 128)],
                                     ch[:, 0:LC], start=(j == 0), stop=False)
                    nc.tensor.matmul(ypsA1[:], r01all[:, ts(jj, 128)],
                                     ch[:, LC:LCF], start=(j == 0), stop=False)
                else:
                    nc.tensor.matmul(ypsB0, r01ball[:, ts(jj, 64)],
                                     ch[:, 0:LC], start=(j == 16), stop=False,
                                     skip_group_check=True)
                    nc.tensor.matmul(ypsB1, r01ball[:, ts(jj, 64)],
                                     ch[:, LC:LCF], start=(j == 16), stop=False,
                                     skip_group_check=True)
            # Dp residual folded into the y accumulation as a diagonal matmul
            yA = pT.tile([DA, LCF], bf16, name="yA", bufs=1)
            yB = pT.tile([DB, LCF], bf16, name="yB", bufs=1)
            for h, (ya_ps, yb_ps) in enumerate([(ypsA0[:], ypsB0), (ypsA1[:], ypsB1)]):
                off = i * LCF + h * LC
                sl = bass.ds(off, LC)
                nc.tensor.matmul(ya_ps, dpdA[:], xi2A[:, sl],
                                 start=False, stop=True, skip_group_check=True)
                nc.tensor.matmul(yb_ps, dpdB[:], xi2B[:, sl],
                                 start=False, stop=True, skip_group_check=True)
                nc.vector.tensor_tensor(yA[:, ts(h, LC)], ya_ps,
                                        zsA[:, sl], op=OP.mult)
                nc.vector.tensor_tensor(yB[:, ts(h, LC)], yb_ps,
                                        zsB[:, sl], op=OP.mult)
            mout = pT.tile([C, LCF], bf16, name="mout", bufs=1)
            for h in range(2):
                mps = mmtile(C, LC, "mps")
                nc.tensor.matmul(mps[:], outwTA[:], yA[:, ts(h, LC)],
                                 start=True, stop=False)
                nc.tensor.matmul(mps[:], outwTB[:], yB[:, ts(h, LC)],
                                 start=False, stop=True)
                nc.scalar.copy(mout[:, ts(h, LC)], mps[:])
            # Z = xdq + mout^T computed wholly in PSUM: transpose matmul and
            # an identity matmul accumulate into one bank; Scalar copies out.
            # zps lives in the mm pool so the single tp bank stays dedicated
            # to tps2 and the next iteration's front-end isn't tied to this
            # iteration's tail.
            for r0 in range(0, 16, 4):
                zps = pmm.tile([WQ, 4 * C], f32, name="zps", tag="mm")
                zps3 = zps.rearrange("p (r c) -> p r c", r=4)
                for k in range(4):
                    r = r0 + k
                    nc.tensor.matmul(zps3[:, k, :], mout[:, ts(r, WQ)],
                                     identb[:], start=True, stop=False)
                    nc.tensor.matmul(zps3[:, k, :], identb[0:WQ, 0:WQ],
                                     xdq3[:, :, 16 * i + r],
                                     start=False, stop=True)
                hq0 = 16 * i + r0
                nc.scalar.activation(Z3[:, :, hq0:hq0 + 4],
                                     zps3.transpose([0, 2, 1]), AF.Copy)
        pT.release()
        pC.release()
        pE.release()
        pG.release()

        # =============== Phase G: IDCT contribution ===============
        pH = tc.alloc_tile_pool(name="pH", bufs=1)
        # t7[hq, c, W] = sum_wq Z[wq, c, hq] * Mw_q[wq, W]  (per-c matmul)
        t7 = pH.tile([HQ, C * W], bf16)
        t7_3 = t7.rearrange("p (c w) -> p c w", c=C)
        for ci, c0 in enumerate(range(0, C, 4)):
            t7ps = tptile(HQ, 4 * W, "t7ps")
            t7ps3 = t7ps.rearrange("p (c w) -> p c w", c=4)
            for k in range(4):
                nc.tensor.matmul(t7ps3[:, k, :], Z3[:, c0 + k, :],
                                 mwq64[64:128, :], start=True, stop=True)
            if ci % 2 == 0:
                nc.scalar.activation(t7_3[:, c0:c0 + 4, :], t7ps3[:, :, :],
                                     AF.Copy)
            else:
                nc.vector.tensor_copy(t7_3[:, c0:c0 + 4, :], t7ps3[:, :, :])
        ctr_h = contrib
        pSo = tc.alloc_tile_pool(name="pSo", bufs=3)
        for i in range(24):
            cps = mmtile(H, LC, "cps")
            nc.tensor.matmul(cps[:], mhq[:], t7[:, ts(i, LC)], start=True, stop=True)
            csb = pSo.tile([H, LC], bf16, name="csb")
            if i % 2 == 0:
                nc.scalar.activation(csb[:], cps[:], AF.Copy)
            else:
                nc.vector.tensor_copy(csb[:], cps[:])
            nc.sync.dma_start(
                ctr_h[:, ts(i, 4), :],
                csb[:].rearrange("h (c w) -> h c w", c=4))
        pSo.release()
        pH.release()
        pD_.release()
        ptp.release()
        ppy.release()
        pmm.release()
        consts.release()

    nc.compile()
    return nc


def _host_inputs(inputs):
    """Build the 8 per-core input maps."""
    x = inputs["x"]
    ln_w, ln_b = inputs["ln_w"], inputs["ln_b"]
    Mh = _dct_mat(H)
    Mw = _dct_mat(W)
    ident = np.eye(128, dtype=np.float32)
    # within-tile partition order p = 8*s + r (r = d offset, s = state):
    # makes row replication p -> p%8 a contiguous-partition doubling
    s01 = np.zeros((S, 128), np.float32)
    p0164 = np.zeros((128, 128 * 8), np.float32)
    r01all = np.zeros((128, 128 * 16), np.float32)
    r01ball = np.zeros((128, 64 * 8), np.float32)
    for p in range(128):
        s01[p // 8, p] = 1.0
        for j in range(8):
            p0164[8 * j + p % 8, 128 * j + p] = 1.0
            p0164[64 + 8 * j + p % 8, 128 * j + p] = 1.0
        for j in range(16):
            r01all[p, 128 * j + 8 * j + p % 8] = 1.0
        for j in range(8):
            r01ball[p, 64 * j + 8 * j + p % 8] = 1.0
    in_maps = []
    for k in range(8):
        b, q = k // 4, k % 4
        h0 = (q // 2) * HQ
        w0 = (q % 2) * WQ
        in_w2 = (inputs["in_w"][q] * ln_w[None, :]).astype(np.float32)
        bias_e = (inputs["in_w"][q] @ ln_b).astype(np.float32)
        # conv folded into in_proj: tap k' applies weight conv_w[:, 3-k'] to
        # positions shifted back by k'
        convw = inputs["conv_w"][q].astype(np.float32)        # [D, 4]
        convb = inputs["conv_b"][q].astype(np.float32)        # [D]
        w_rev = convw[:, ::-1]
        WX = in_w2[:D]                                        # [D, C]
        inwtap = np.concatenate(
            [(WX * w_rev[:, kk][:, None]).T for kk in range(KCONV)], axis=1)
        bconv = bias_e[:D] * convw.sum(1) + convb
        bc3 = np.stack(
            [bias_e[:D] * w_rev[:, :t + 1].sum(1) + convb
             for t in range(KCONV - 1)], axis=1)              # [D, 3]
        xpw80 = np.zeros((D, 80), np.float32)
        xpwT = inputs["xp_w"][q].T
        xpw80[:, 0:RK] = xpwT[:, 0:RK]
        xpw80[:, 32:32 + S] = xpwT[:, RK:RK + S]
        xpw80[:, 64:64 + S] = xpwT[:, RK + S:RK + 2 * S]
        A = (-np.exp(inputs["A_log"][q])).astype(np.float32)  # [D, S]
        acol = np.zeros((128, NT), np.float32)
        for j in range(NT):
            for p in range(128):
                acol[p, j] = A[j * 8 + p % 8, p // 8]
        m = {
            "xb": x[b],
            "xbh": np.ascontiguousarray(x[b].transpose(1, 0, 2)),
            "mhqT": Mh[h0:h0 + HQ, :].T,
            "mwqT": Mw[w0:w0 + WQ, :].T,
            "mhq": Mh[h0:h0 + HQ, :],
            "mwq": Mw[w0:w0 + WQ, :],
            "ident": ident,
            "inwtap": inwtap,
            "inwz": in_w2[D:].T,
            "biasz": bias_e[D:, None],
            "bconv": bconv[:, None],
            "bc3": bc3,
            "xpwT": xpw80,
            "dtwT": inputs["dt_w"][q].T,
            "dtb": inputs["dt_b"][q][:, None],
            "acol": acol,
            "dpdA": np.diag(inputs["Dp"][q][:DA]),
            "dpdB": np.diag(inputs["Dp"][q][DA:]),
            "outwT": inputs["out_w"][q].T,
            "p0164": p0164,
            "s01": s01,
            "r01all": r01all,
            "r01ball": r01ball,
        }
        import ml_dtypes
        bf = ["inwtap", "inwz", "xpwT", "dtwT", "outwT", "p0164", "s01",
              "dpdA", "dpdB",
              "r01all", "r01ball", "xb", "xbh", "mhqT", "mwqT", "mhq", "mwq",
              "contrib"]
        in_maps.append({
            kk: np.ascontiguousarray(np.asarray(
                vv, ml_dtypes.bfloat16 if kk in bf else np.float32))
            for kk, vv in m.items()})
    return in_maps


def kernel(**inputs):
    from concourse import bass_utils
    inputs = {k: np.asarray(v) for k, v in inputs.items()}
    if "nc" not in _BUILT:
        _BUILT["nc"] = _build_nc()
    nc = _BUILT["nc"]
    in_maps = _host_inputs(inputs)
    res = bass_utils.run_bass_kernel_spmd(nc, in_maps, core_ids=list(range(8)))
    out = np.zeros((B, C, H, W), np.float32)
    for k in range(8):
        out[k // 4] += np.asarray(
            res.results[k]["contrib"], np.float32).transpose(1, 0, 2)
    return out


if __name__ == "__main__":
    # smoke: random inputs, shape check only
    rng = np.random.default_rng(0)
    demo = {
        "x": rng.standard_normal((B, C, H, W), np.float32),
        "ln_w": np.ones(C, np.float32), "ln_b": np.zeros(C, np.float32),
        "in_w": rng.standard_normal((4, 2 * D, C), np.float32) * 0.02,
        "conv_w": rng.standard_normal((4, D, KCONV), np.float32) * 0.02,
        "conv_b": np.zeros((4, D), np.float32),
        "xp_w": rng.standard_normal((4, RK + 2 * S, D), np.float32) * 0.02,
        "dt_w": rng.standard_normal((4, D, RK), np.float32) * 0.02,
        "dt_b": np.full((4, D), -4.0, np.float32),
        "A_log": np.tile(np.log(np.arange(1, S + 1, dtype=np.float32)), (4, D, 1)),
        "Dp": np.ones((4, D), np.float32),
        "out_w": rng.standard_normal((4, C, D), np.float32) * 0.02,
    }
    out = kernel(**demo)
    print("kernel output:", out.shape, out.dtype)

